# revision 43
# baseline (speedup 1.0000x reference)
"""BWGNN_Hetero Trainium2 kernel (8 NeuronCores, SPMD).

Math restructure of the reference:
  - poly_conv with THETAS (Bernstein, D=2) shares the Krylov basis
    P0 = h, P1 = L~ P0, P2 = L~ P1 across all three thetas, where
    L~ x = x - dinv * segsum((x*dinv)[src], dst).
  - concat(h_theta) @ W3 collapses to sum_k P_k @ V_k with
    V_k = sum_i THETA[i][k] * W3[i*H:(i+1)*H] (host-precomputed).
  - out = leaky(sum_r sum_k P_k^r @ V_k + 2*b3)

Sharding: nodes split 8 ways (6250/core, padded to 6272 = 49*128).
The all-gathered feature table packs TWO nodes per 256-byte row
([25088, 128] bf16): local node position p -> row 64*(p//128) +
(p%128)//2, column-half p%2. int16 gather indices cover the whole
table; edges split by the src's column-half (== src position parity
== src node-id parity, enforced by the balance permutation giving
each block 32 even-id + 32 odd-id slots).

Aggregation processes 128-node SUPERBLOCK pairs: gathered subtiles of
the two 64-node blocks are interleaved [A_s|B_s] so each [128,128]
stationary load feeds one [128,128] one-hot matmul producing both
blocks' aggregates in opposite PSUM quadrants (2 subtiles per
LoadStationary).

Gather calls are one (pair, half) = 1024 idxs each: the SWDGE ring
holds 128 descriptors per SDMA engine, so 1024-idx calls (64/engine
+ sem) are the largest that decode without deadlock; the Q7
descriptor-push rate (~8.6us/call, 4 queues in parallel) is the hard
floor for the prop phases, which run the gpsimd engine at ~100%.

Overlap structure: MLP leaky-ReLUs run on the otherwise-idle ACT
engine; write_scaled is PE-transpose + one DVE broadcast-scale per
superblock, with 4 superblocks batched into ONE 3D DMA (DRAM addr =
64p + 8192j + f is affine in partition p). Each AllGather is split
into two chunk-aligned halves (table layout [8 x rows 0:1536 | 8 x
rows 1536:3200]) so the first half launches while the second is
still being produced, and mid-kernel AG triggers are emitted after
group 2 of the following prop so their input-waits never stall the
gather queue head. Bulk constant loads (idx/dstoff/dinv) are
deferred until after AG(0,0) is triggered; the final output stage is
fused per-pair into the last prop.
"""

import math
import os
import sys

import ml_dtypes
import numpy as np

for _p in ("/opt/trn_rl_repo", "/root/.axon_site/_ro/trn_rl_repo"):
    if os.path.isdir(_p) and _p not in sys.path:
        sys.path.insert(0, _p)

import concourse.bacc as bacc
import concourse.bass as bass
import concourse.mybir as mybir
import concourse.tile as tile
from concourse.bass_utils import run_bass_kernel_spmd

F32 = mybir.dt.float32
BF16 = mybir.dt.bfloat16
I16 = mybir.dt.int16


class Cfg:
    def __init__(self, N=50000, F=128, H=64, NCORES=8):
        self.N = N
        self.F = F
        self.H = H
        self.NCORES = NCORES
        self.NPC = N // NCORES              # real nodes per core
        self.BLK = 64                       # dst nodes per block
        self.NPAD = 6400                    # padded nodes per core (50*128)
        assert self.NPAD % 128 == 0 and self.NPAD >= self.NPC
        self.NBLK = self.NPAD // self.BLK   # 100
        self.NPAIR = self.NBLK // 2         # 50 superblocks of 128 nodes
        self.TPC = self.NPAD // 2           # table rows per core (3200)
        self.TROWS = self.TPC * NCORES      # 25600 (< 32768: int16 idx ok)
        self.GP = 1                         # pairs per gather call


def _calc_thetas(d=2):
    thetas = []
    for i in range(d + 1):
        p1 = np.zeros(i + 1)
        p1[i] = 0.5 ** i
        m = d - i
        p2 = np.array([math.comb(m, j) * (-0.5) ** j for j in range(m + 1)])
        c = np.convolve(p1, p2)
        beta = math.factorial(i) * math.factorial(d - i) / math.factorial(d + 1)
        thetas.append(c / beta)
    return np.stack(thetas)  # [3, 3] increasing power


THETAS = _calc_thetas(2)


def _pack_idx16(flat):
    """Q7 layout: idx i -> partition i%16, free i//16; replicated x8 groups."""
    n = len(flat)
    arr = flat.astype(np.int16).reshape(n // 16, 16).T  # [16, n/16]
    return np.tile(arr, (8, 1))  # [128, n/16]


def _greedy_balance(cnt4, parity, cfg, hard=False):
    """Assign nodes to blocks minimizing max per-(rel, src-col-half) count.

    Soft pass: slot parity prefers the node's parity, overflows flip.
    Hard pass (parity = previous pass's realized col-half): slot parity
    is fixed, so cnt4 (keyed on that col-half) is exact.
    """
    c = cfg
    order = np.argsort(-cnt4.sum(1), kind="stable")
    blk_cnt = np.zeros((c.NBLK, 4), np.int64)
    blk_n = np.zeros((c.NBLK, 2), np.int64)
    pos = np.zeros(len(cnt4), np.int64)
    BIG = 1 << 40
    for n in order:
        pi = int(parity[n])
        score = (blk_cnt + cnt4[n]).max(axis=1) + blk_cnt.max(axis=1)
        if hard:
            score[blk_n[:, pi] >= c.BLK // 2] = BIG
        else:
            score[blk_n.sum(1) >= c.BLK] = BIG
        b = int(np.argmin(score))
        if not hard and blk_n[b, pi] >= c.BLK // 2:
            pi = 1 - pi
        pos[n] = b * c.BLK + 2 * blk_n[b, pi] + pi
        blk_n[b, pi] += 1
        blk_cnt[b] += cnt4[n]
    return pos


def _prep_relation(src, dst, cfg, POS, SROW, SH):
    """Group one relation's edges by (dst-owner core, pair, src-half, block)."""
    c = cfg
    deg = np.bincount(dst, minlength=c.N).astype(np.float32)
    dinv = np.clip(deg, 1.0, None) ** -0.5
    per_core = []
    lh = 0
    for core in range(c.NCORES):
        m = (dst // c.NPC) == core
        s_row = SROW[src[m]]
        s_h = SH[src[m]]
        d_loc = POS[dst[m]] - core * c.NPAD
        pair = d_loc // 128
        ab = (d_loc % 128) // c.BLK
        off = (d_loc % 128).astype(np.float32)
        key = (pair * 2 + s_h) * 2 + ab
        order = np.lexsort((s_row, key))
        key_s, row_s, off_s = key[order], s_row[order], off[order]
        counts = np.bincount(key_s, minlength=c.NPAIR * 4)
        starts = np.concatenate(([0], np.cumsum(counts)[:-1]))
        within = np.arange(len(key_s)) - starts[key_s]
        lh = max(lh, int(counts.max()) if len(counts) else 0)
        per_core.append((key_s, within, row_s, off_s))
    return per_core, dinv, lh


def _finalize_relation(slots_pc, cfg, NS):
    """Build per-core packed idx + dstoff tensors for one relation."""
    c = cfg
    L = NS * 128
    out = []
    for core in range(c.NCORES):
        key_s, within, row_s, off_s = slots_pc[core]
        # pad slots get spread (distinct) rows: duplicate same-address
        # descriptors serialize the SDMA engines (~12x slower packets)
        idx_slots = np.broadcast_to(
            (np.arange(L) * 97 + 13) % c.TROWS, (c.NPAIR * 4, L)).copy()
        off_slots = np.full((c.NPAIR * 4, L), 255.0, np.float32)
        idx_slots[key_s, within] = row_s
        off_slots[key_s, within] = off_s
        # interleaved stream per (pair, h): subtile t = 2*s + ab
        sl = idx_slots.reshape(c.NPAIR, 2, 2, NS, 128)
        inter = sl.transpose(0, 1, 3, 2, 4)  # [q, h, s, ab, 128]
        parts = []
        for g in range(c.NPAIR // c.GP):
            for h in range(2):
                for q in range(g * c.GP, (g + 1) * c.GP):
                    parts.append(_pack_idx16(inter[q, h].reshape(-1)))
        idx_packed = np.concatenate(parts, axis=1)
        # dstoff col ((q*2+h)*2+ab)*NS + s = offs of subtile rows [128]
        do = (off_slots.reshape(c.NPAIR * 4, NS, 128)
              .transpose(2, 0, 1)
              .reshape(128, c.NPAIR * 4 * NS)
              .astype(ml_dtypes.bfloat16))
        out.append((idx_packed, do))
    return out


def _build(cfg, NS, debug=False):
    """Build the SPMD Bass graph (identical on all cores)."""
    c = cfg
    H = c.H
    NQ = 4
    NSLOT = 12
    GP = c.GP                             # pairs per gather call
    SUBS = GP * 2 * NS                    # subtiles per gather call

    nc = bacc.Bacc("TRN2", target_bir_lowering=False, debug=False,
                   num_devices=c.NCORES, num_swdge_queues=NQ,
                   dynamic_dma_scratch_size=49152)

    dram_in = {}

    def din(name, shape, dtype=F32):
        dram_in[name] = nc.dram_tensor(name, list(shape), dtype,
                                       kind="ExternalInput")
        return dram_in[name]

    IDXCOLS = 2 * c.NPAIR * 2 * NS * 128 // 16
    for r in range(2):
        din(f"featT{r}", (c.F, c.NPAD), BF16)
        din(f"idx{r}", (128, IDXCOLS), I16)
        din(f"dstoff{r}", (128, c.NPAIR * 4 * NS), BF16)
        din(f"W1_{r}", (c.F, H), BF16)
        din(f"W2_{r}", (H, H), BF16)
        din(f"b1_{r}", (H, 1))
        din(f"b2_{r}", (H, 1))
    din("dinv0", (H, c.NPAD), BF16)
    din("dinv1", (H, c.NPAD), BF16)
    din("dinvC0", (128, c.NPAIR))
    din("dinvC1", (128, c.NPAIR))
    din("Vk", (H, 3 * H), BF16)
    din("identb", (128, 128), BF16)
    din("b3bc", (128, H))
    din("iota", (128, 128), BF16)

    out_t = nc.dram_tensor("out", [c.NPAD, H], F32, kind="ExternalOutput")
    if debug:
        dbg = {
            "dbg_P0T": nc.dram_tensor("dbg_P0T", [H, c.NPAD], BF16,
                                      kind="ExternalOutput"),
            "dbg_tab": nc.dram_tensor("dbg_tab", [c.TROWS, 128], BF16,
                                      kind="ExternalOutput"),
            "dbg_P1T": nc.dram_tensor("dbg_P1T", [H, c.NPAD], BF16,
                                      kind="ExternalOutput"),
            "dbg_gt0": nc.dram_tensor("dbg_gt0", [128, 2 * NS * H], BF16,
                                      kind="ExternalOutput"),
            "dbg_gt1": nc.dram_tensor("dbg_gt1", [128, 2 * NS * H], BF16,
                                      kind="ExternalOutput"),
            "dbg_oh": nc.dram_tensor("dbg_oh", [128, 2 * NS * 128], BF16,
                                     kind="ExternalOutput"),
        }
        DBGQ = 4

    rg = [list(range(c.NCORES))]

    # per-call idx column base (in int16 columns) for call (g, h)
    CALL16 = GP * 2 * NS * 128 // 16
    call_base = {(g, h): (g * 2 + h) * CALL16
                 for g in range(c.NPAIR // GP) for h in range(2)}
    assert 2 * (c.NPAIR // GP) * CALL16 == IDXCOLS

    with tile.TileContext(nc) as tc:
        with (
            tc.tile_pool(name="const", bufs=1) as constp,
            tc.tile_pool(name="dram", bufs=1, space="DRAM") as dramp,
            tc.tile_pool(name="feat", bufs=4) as featp,
            tc.tile_pool(name="h1", bufs=3) as h1p,
            tc.tile_pool(name="oh", bufs=3) as ohp,
            tc.tile_pool(name="sc", bufs=4) as scp,
            tc.tile_pool(name="stg", bufs=3) as stgp,
            tc.tile_pool(name="psmlp", bufs=2, space="PSUM") as psmlp,
            tc.tile_pool(name="psagg", bufs=3, space="PSUM") as psagg,
            tc.tile_pool(name="psmisc", bufs=3, space="PSUM") as psmisc,
        ):
            def load_const(name, shape, dtype=F32, eng=None):
                t = constp.tile(list(shape), dtype, name=f"c_{name}")
                (eng or nc.scalar).dma_start(out=t[:], in_=dram_in[name].ap()[:])
                return t

            # essentials for mlp(0)+write_scaled (loaded up front)
            W1 = [load_const(f"W1_{r}", (c.F, H), BF16) for r in range(2)]
            W2 = [load_const(f"W2_{r}", (H, H), BF16) for r in range(2)]
            b1 = [load_const(f"b1_{r}", (H, 1)) for r in range(2)]
            b2 = [load_const(f"b2_{r}", (H, 1)) for r in range(2)]
            dinvC = [load_const(f"dinvC{r}", (128, c.NPAIR)) for r in range(2)]
            identb = load_const("identb", (128, 128), BF16)
            # bulk tensors only needed from the first prop (~150us in) are
            # loaded via deferred_loads() after AG(0,0) is triggered so the
            # DMA queues stay clear for featT/agin during startup.
            dinv = [None, None]
            dstoff = [None, None]
            idxsb = [None, None]
            Vk = b3bc = iota = None

            def deferred_loads():
                nonlocal Vk, b3bc, iota
                for r in range(2):
                    idxsb[r] = load_const(f"idx{r}", (128, IDXCOLS), I16)
                    dinv[r] = load_const(f"dinv{r}", (H, c.NPAD), BF16,
                                         eng=nc.sync)
                    dstoff[r] = load_const(f"dstoff{r}",
                                           (128, c.NPAIR * 4 * NS), BF16,
                                           eng=nc.sync)
                Vk = load_const("Vk", (H, 3 * H), BF16, eng=nc.sync)
                b3bc = load_const("b3bc", (128, H), eng=nc.sync)
                iota = load_const("iota", (128, 128), BF16, eng=nc.sync)

            # ---- persistent SBUF state ----
            P0T = [constp.tile([H, c.NPAD], BF16, name=f"P0T{r}") for r in range(2)]
            P1T = [constp.tile([H, c.NPAD], BF16, name=f"P1T{r}") for r in range(2)]
            outaccT = constp.tile([H, c.NPAD], BF16, name="outaccT")
            GTS = [constp.tile([128, SUBS, H], BF16, name=f"gtslot{i}")
                   for i in range(NSLOT)]

            # ---- internal DRAM ----
            agin = [[dramp.tile([c.TPC, 128], BF16, name=f"agin{r}_{hp}")
                     for hp in range(2)] for r in range(2)]
            table = [[dramp.tile([c.TROWS, 128], BF16, name=f"table{r}_{hp}")
                      for hp in range(2)] for r in range(2)]

            CHUNKS = []
            pos2 = 0
            while pos2 < c.NPAD:
                w = min(512, c.NPAD - pos2)
                CHUNKS.append((pos2, w))
                pos2 += w

            def mlp(r):
                for (p0, w) in CHUNKS:
                    ft = featp.tile([c.F, 512], BF16, name="ft")
                    nc.sync.dma_start(out=ft[:, :w],
                                      in_=dram_in[f"featT{r}"].ap()[:, p0 : p0 + w])
                    ps1 = psmlp.tile([H, 512], F32, name="ps1", tag="mlp")
                    nc.tensor.matmul(ps1[:, :w], W1[r][:], ft[:, :w],
                                     start=True, stop=True)
                    h1t = h1p.tile([H, 512], BF16, name="h1t")
                    nc.scalar.activation(h1t[:, :w], ps1[:, :w],
                                         mybir.ActivationFunctionType.Lrelu,
                                         bias=b1[r][:], alpha=0.01)
                    ps2 = psmlp.tile([H, 512], F32, name="ps2", tag="mlp")
                    nc.tensor.matmul(ps2[:, :w], W2[r][:], h1t[:, :w],
                                     start=True, stop=True)
                    nc.scalar.activation(P0T[r][:, p0 : p0 + w], ps2[:, :w],
                                         mybir.ActivationFunctionType.Lrelu,
                                         bias=b2[r][:], alpha=0.01)

            def write_scaled(PT, r, agin_t):
                """agin row 64*(p//128)+(p%128)//2, col-half p%2 <- dinv*PT.

                DRAM address is affine in partition: addr_el = 64p + 8192j + f
                (j = superblock within chunk), so a chunk's 2-4 superblocks
                go out in ONE 3D DMA instead of 4 small ones.
                """
                for (p0, w) in CHUNKS:
                    ns = w // 128
                    stg4 = stgp.tile([128, 4, H], BF16, name="stg4")
                    for si in range(ns):
                        s = p0 // 128 + si
                        tp = psmisc.tile([128, H], BF16, name="tp", tag="misc")
                        nc.tensor.transpose(
                            tp[:], PT[:, s * 128 : s * 128 + 128], identb[:H, :H])
                        nc.vector.tensor_tensor(
                            out=stg4[:, si, :], in0=tp[:],
                            in1=dinvC[r][:, s : s + 1].to_broadcast((128, H)),
                            op=mybir.AluOpType.mult)
                    out_ap = bass.AP(
                        tensor=agin_t[:].tensor, offset=p0 * 64,
                        ap=[[64, 128], [8192, ns], [1, H]])
                    nc.sync.dma_start(out=out_ap, in_=stg4[:, 0:ns, :])

            def allgather(r, hp):
                ofs = 0
                for (st, ln) in ((0, 1536), (1536, c.TPC - 1536)):
                    nc.gpsimd.collective_compute(
                        "AllGather",
                        mybir.AluOpType.bypass,
                        replica_groups=rg,
                        ins=[agin[r][hp][st : st + ln, :].opt()],
                        outs=[table[r][hp][ofs : ofs + c.NCORES * ln, :].opt()],
                    )
                    ofs += c.NCORES * ln

            def prop(r, hop, after2=None):
                """hop=1: P1T = L~ P0T. hop=2: fused output accumulation.

                after2() is emitted after group 2's calls so a following
                collective trigger doesn't block the gather queue head
                while its input is still being written."""
                tab = table[r][hop - 1]
                PTin = P0T[r] if hop == 1 else P1T[r]

                def issue_call(g, h):
                    b16 = call_base[(g, h)]
                    gt = GTS[(g * 2 + h) % NSLOT]
                    src_ap = tab[:, 0:H] if h == 0 else tab[:, H : 2 * H]
                    n_i = GP * 2 * NS * 128
                    _dma_gather_narrow(
                        nc.gpsimd, gt[:, 0:SUBS, :], src_ap,
                        idxsb[r][:, b16 : b16 + n_i // 16],
                        n_i, n_i, H, 2 * H,
                        queue_num=(g * 2 + h) % NQ)
                    return gt

                def do_pair(q, gts):
                    bs = slice(q * 128, (q + 1) * 128)
                    oh = ohp.tile([128, 2 * NS, 128], BF16, name="oh")
                    for h in range(2):
                        for ab in range(2):
                            col = ((q * 2 + h) * 2 + ab) * NS
                            nc.vector.tensor_tensor(
                                out=oh[:, h * NS : (h + 1) * NS,
                                       ab * 64 : ab * 64 + 64],
                                in0=dstoff[r][:, col : col + NS][:, :, None]
                                    .to_broadcast((128, NS, 64)),
                                in1=iota[:][:, None, ab * 64 : ab * 64 + 64]
                                    .to_broadcast((128, NS, 64)),
                                op=mybir.AluOpType.is_equal,
                            )
                    if debug and r == 0 and hop == 1 and q == DBGQ:
                        nc.sync.dma_start(out=dbg["dbg_gt0"].ap()[:],
                                          in_=gts[0][:, :, :])
                        nc.sync.dma_start(out=dbg["dbg_gt1"].ap()[:],
                                          in_=gts[1][:, :, :])
                        nc.sync.dma_start(out=dbg["dbg_oh"].ap()[:],
                                          in_=oh[:, :, :])
                    ql = q % GP
                    agg = psagg.tile([128, 128], F32, name="agg")
                    k = 0
                    for h in range(2):
                        base = ql * 2 * NS
                        for s in range(NS):
                            nc.tensor.matmul(
                                agg[:],
                                gts[h][:, base + 2 * s : base + 2 * s + 2, :],
                                oh[:, h * NS + s, :],
                                start=(k == 0),
                                stop=(k == 2 * NS - 1),
                            )
                            k += 1
                    # tmp = dinv * agg (combine the two useful quadrants)
                    tmp = scp.tile([H, 128], BF16, name="tmp")
                    dv = dinv[r][:, bs]
                    nc.vector.tensor_tensor(out=tmp[:, 0:64], in0=agg[0:64, 0:64],
                                            in1=dv[:, 0:64],
                                            op=mybir.AluOpType.mult)
                    nc.vector.tensor_tensor(out=tmp[:, 64:128],
                                            in0=agg[64:128, 64:128],
                                            in1=dv[:, 64:128],
                                            op=mybir.AluOpType.mult)
                    if hop == 1:
                        nc.vector.tensor_tensor(out=P1T[r][:, bs],
                                                in0=PTin[:, bs], in1=tmp[:],
                                                op=mybir.AluOpType.subtract)
                    else:
                        p2 = scp.tile([H, 128], BF16, name="p2")
                        nc.vector.tensor_tensor(out=p2[:], in0=PTin[:, bs],
                                                in1=tmp[:],
                                                op=mybir.AluOpType.subtract)
                        op_ps = psmisc.tile([H, 128], F32, name="opps", tag="misc")
                        nc.tensor.matmul(op_ps[:], Vk[:, 0:H], P0T[r][:, bs],
                                         start=True, stop=False)
                        nc.tensor.matmul(op_ps[:], Vk[:, H : 2 * H],
                                         P1T[r][:, bs], start=False, stop=False)
                        nc.tensor.matmul(op_ps[:], Vk[:, 2 * H : 3 * H], p2[:],
                                         start=False, stop=True)
                        if r == 0:
                            nc.vector.tensor_copy(out=outaccT[:, bs],
                                                  in_=op_ps[:])
                        else:
                            nc.vector.tensor_add(out=outaccT[:, bs],
                                                 in0=outaccT[:, bs],
                                                 in1=op_ps[:])
                            final_pair(q)

                LOOK = 4
                NG = c.NPAIR // GP
                gts_by_g = {}
                for g in range(min(LOOK + 1, NG)):
                    gts_by_g[g] = [issue_call(g, 0), issue_call(g, 1)]
                for g in range(NG):
                    g2 = g + LOOK + 1
                    if g2 < NG:
                        gts_by_g[g2] = [issue_call(g2, 0), issue_call(g2, 1)]
                    if g == 2 and after2 is not None:
                        after2()
                    for q in range(g * GP, (g + 1) * GP):
                        do_pair(q, gts_by_g[g])
                    gts_by_g.pop(g)

            def final_pair(q):
                bs = slice(q * 128, (q + 1) * 128)
                tp = psmisc.tile([128, H], BF16, name="tpo", tag="misc")
                nc.tensor.transpose(tp[:], outaccT[:, bs], identb[:H, :H])
                t = scp.tile([128, H], F32, name="tf")
                nc.vector.tensor_tensor(out=t[:], in0=tp[:], in1=b3bc[:],
                                        op=mybir.AluOpType.add)
                so = stgp.tile([128, H], F32, name="so")
                nc.vector.scalar_tensor_tensor(
                    out=so[:], in0=t[:], scalar=0.01, in1=t[:],
                    op0=mybir.AluOpType.mult, op1=mybir.AluOpType.max)
                nc.sync.dma_start(out=out_t.ap()[bs, :], in_=so[:])

            mlp(0)
            write_scaled(P0T[0], 0, agin[0][0])
            allgather(0, 0)
            deferred_loads()
            if debug:
                nc.sync.dma_start(out=dbg["dbg_P0T"].ap()[:], in_=P0T[0][:])
                nc.sync.dma_start(out=dbg["dbg_tab"].ap()[:],
                                  in_=table[0][0][:])
            mlp(1)
            write_scaled(P0T[1], 1, agin[1][0])
            prop(0, 1, after2=lambda: allgather(1, 0))
            if debug:
                nc.sync.dma_start(out=dbg["dbg_P1T"].ap()[:], in_=P1T[0][:])
            write_scaled(P1T[0], 0, agin[0][1])
            prop(1, 1, after2=lambda: allgather(0, 1))
            write_scaled(P1T[1], 1, agin[1][1])
            prop(0, 2, after2=lambda: allgather(1, 1))
            prop(1, 2)

    nc.compile()
    return nc


def _prepare(inputs, cfg):
    c = cfg
    H = c.H
    W3 = inputs["W3"]
    V = np.zeros((H, 3 * H), np.float32)
    for k in range(3):
        acc = np.zeros((H, H), np.float64)
        for i in range(3):
            acc += THETAS[i][k] * W3[i * H : (i + 1) * H].astype(np.float64)
        V[:, k * H : (k + 1) * H] = acc.astype(np.float32)

    srcs = [np.asarray(inputs["src_r1"]).astype(np.int64),
            np.asarray(inputs["src_r2"]).astype(np.int64)]
    dsts = [np.asarray(inputs["dst_r1"]).astype(np.int64),
            np.asarray(inputs["dst_r2"]).astype(np.int64)]

    # Parity-constrained balance: node n only takes a position p with
    # p%2 == n%2, so a source's table column-half (p%2) is known up front.
    POS = np.zeros(c.N, np.int64)
    H1 = np.zeros(c.N, np.int64)
    for it in range(2):
        hsrc = [(srcs[r] % 2) if it == 0 else H1[srcs[r]] for r in range(2)]
        for core in range(c.NCORES):
            cnt4 = np.zeros((c.NPC, 4), np.int64)
            for r in range(2):
                m = (dsts[r] // c.NPC) == core
                d_loc = dsts[r][m] - core * c.NPC
                np.add.at(cnt4, (d_loc, 2 * r + hsrc[r][m]), 1)
            gids = np.arange(core * c.NPC, (core + 1) * c.NPC)
            pos = _greedy_balance(cnt4, (gids % 2) if it == 0 else H1[gids],
                                  c, hard=(it == 1))
            POS[gids] = core * c.NPAD + pos
        H1 = POS % 2

    # node global padded position -> table row / column-half.
    # Table layout (chunked AllGather): [8 x rows 0:1536 | 8 x rows 1536:TPC]
    loc = POS % c.NPAD
    core_of = POS // c.NPAD
    lr = (loc // 128) * c.BLK + (loc % 128) // 2
    SROW = np.where(lr < 1536, core_of * 1536 + lr,
                    c.NCORES * 1536 + core_of * (c.TPC - 1536) + (lr - 1536))
    SH = loc % 2

    rels = []
    LH = 128
    for r in range(2):
        slots_pc, dinv, lh = _prep_relation(srcs[r], dsts[r], c, POS, SROW, SH)
        rels.append((slots_pc, dinv))
        LH = max(LH, lh)
    NS = -(-LH // 128)

    fin = [_finalize_relation(rels[r][0], c, NS) for r in range(2)]

    iota = np.broadcast_to(np.arange(128, dtype=np.float32), (128, 128))
    ident = np.eye(128, dtype=np.float32)

    in_maps = []
    for core in range(c.NCORES):
        m = {}
        lpos_all = POS[core * c.NPC : (core + 1) * c.NPC] - core * c.NPAD
        dvT = np.zeros((2, H, c.NPAD), np.float32)
        for r in range(2):
            idx_packed, do = fin[r][core]
            m[f"idx{r}"] = idx_packed
            m[f"dstoff{r}"] = do
            feat = np.asarray(inputs["feat_r1" if r == 0 else "feat_r2"],
                              np.float32)
            ft = np.zeros((c.F, c.NPAD), np.float32)
            ft[:, lpos_all] = feat[core * c.NPC : (core + 1) * c.NPC].T
            m[f"featT{r}"] = ft.astype(ml_dtypes.bfloat16)
            dvT[r][:, lpos_all] = np.broadcast_to(
                rels[r][1][core * c.NPC : (core + 1) * c.NPC], (H, c.NPC))
            suf = "_r1" if r == 0 else "_r2"
            m[f"W1_{r}"] = np.asarray(inputs[f"W1{suf}"],
                                      np.float32).astype(ml_dtypes.bfloat16)
            m[f"W2_{r}"] = np.asarray(inputs[f"W2{suf}"],
                                      np.float32).astype(ml_dtypes.bfloat16)
            m[f"b1_{r}"] = np.asarray(inputs[f"b1{suf}"], np.float32).reshape(H, 1)
            m[f"b2_{r}"] = np.asarray(inputs[f"b2{suf}"], np.float32).reshape(H, 1)
        m["dinv0"] = dvT[0].astype(ml_dtypes.bfloat16)
        m["dinv1"] = dvT[1].astype(ml_dtypes.bfloat16)
        m["dinvC0"] = np.ascontiguousarray(dvT[0][0].reshape(c.NPAIR, 128).T)
        m["dinvC1"] = np.ascontiguousarray(dvT[1][0].reshape(c.NPAIR, 128).T)
        m["Vk"] = V.astype(ml_dtypes.bfloat16)
        m["b3bc"] = np.broadcast_to(
            2.0 * np.asarray(inputs["b3"], np.float32), (128, H)).copy()
        m["identb"] = ident.astype(ml_dtypes.bfloat16)
        m["iota"] = iota.astype(ml_dtypes.bfloat16).copy()
        in_maps.append(m)
    return in_maps, NS, POS


def _dma_gather_narrow(gp, out_ap, in_ap, idxs_ap, num_idxs, num_idxs_reg,
                       elem_size, elem_step, queue_num=0):
    """bass.BassGpSimd.dma_gather clone allowing elem_size_bytes % 256 != 0.

    The Q7 kernel (dma_gather.cpp gen_descs, non-transpose HBM path) supports
    any payload length; only the row STRIDE must encode as stride_bytes_256.
    Used to gather 128B bf16 rows from a 256B-strided table.
    """
    import concourse.ap_utils as ap_utils
    assert idxs_ap.dtype == I16
    assert in_ap.dtype == out_ap.dtype
    assert in_ap.space == bass.MemorySpace.DRAM
    assert idxs_ap.space == bass.MemorySpace.SBUF
    assert out_ap.space == bass.MemorySpace.SBUF
    assert ap_utils.ap_is_contiguous(out_ap.ap[1:])
    assert ap_utils.ap_is_contiguous(idxs_ap.ap[1:])
    assert in_ap.ap[0][0] == elem_step
    assert in_ap.ap[-1][1] == elem_size
    assert out_ap.ap[-1][1] == elem_size
    assert out_ap.ap[0][1] * out_ap.ap[1][1] * 1 >= num_idxs
    stride_bytes = elem_step * mybir.dt.size(in_ap.dtype)
    assert stride_bytes % 256 == 0 and stride_bytes // 256 < 256
    _in_ap = gp.lower_ap_dma(in_ap, for_custom_bir_dma=True)
    _idxs_ap = gp.lower_ap(idxs_ap)
    _out_ap = gp.lower_ap(out_ap)
    return gp.add_instruction(
        mybir.InstDMAGatherAnt(
            name=gp.bass.get_next_instruction_name(),
            ins=[*_in_ap, _idxs_ap, gp.lower_val_access(gp.to_reg(num_idxs_reg))],
            outs=[_out_ap],
            transpose=False,
            num_idxs=num_idxs,
            elem_size=elem_size,
            stride_bytes_256=stride_bytes // 256,
            gen_mode=0,
            single_packet=True,
            queue_num=queue_num,
            sbuf_tokens_per_rank=0,
            sbuf_free_dim_per_rank=0,
            sbuf_free_dim_pad_per_rank=0,
            sbuf_byte_offset=0,
        )
    )


_CACHE = {}


def _install_profile_shim():
    """Provide antenv.axon_hooks (missing in this image) so trace=True works."""
    try:
        from antenv.axon_hooks import get_axon_ntff_profile_hook  # noqa: F401
        return
    except ImportError:
        pass
    import types

    import antenv
    try:
        from trn_agent_boot.trn_boot import _ntff_profile_via_ctypes
        hook = _ntff_profile_via_ctypes("/opt/axon/libaxon_pjrt.so")
    except Exception:
        hook = None
    mod = types.ModuleType("antenv.axon_hooks")
    mod._hook = hook
    mod.get_axon_ntff_profile_hook = lambda: mod._hook

    def _set(h):
        mod._hook = h

    mod.set_axon_ntff_profile_hook = _set
    sys.modules["antenv.axon_hooks"] = mod
    antenv.axon_hooks = mod


def _run(inputs, trace=False, **kw):
    if trace:
        _install_profile_shim()
    cfg = Cfg(N=int(np.asarray(inputs["feat_r1"]).shape[0]))
    in_maps, NS, POS = _prepare(inputs, cfg)
    key = (cfg.N, NS, bool(kw.pop("debug", False)))
    if key not in _CACHE:
        _CACHE[key] = _build(cfg, NS, debug=key[2])
    nc = _CACHE[key]
    res = run_bass_kernel_spmd(nc, in_maps, core_ids=list(range(cfg.NCORES)),
                               trace=trace, **kw)
    outs = []
    for core in range(cfg.NCORES):
        lpos = POS[core * cfg.NPC : (core + 1) * cfg.NPC] - core * cfg.NPAD
        outs.append(np.asarray(res.results[core]["out"])[lpos])
    full = np.concatenate(outs, axis=0)
    return full, res


def kernel(**inputs):
    full, _ = _run(inputs, trace=False)
    return full



# revision 47
# speedup vs baseline: 1.0184x; 1.0184x over previous
"""BWGNN_Hetero Trainium2 kernel (8 NeuronCores, SPMD).

Math restructure of the reference:
  - poly_conv with THETAS (Bernstein, D=2) shares the Krylov basis
    P0 = h, P1 = L~ P0, P2 = L~ P1 across all three thetas, where
    L~ x = x - dinv * segsum((x*dinv)[src], dst).
  - concat(h_theta) @ W3 collapses to sum_k P_k @ V_k with
    V_k = sum_i THETA[i][k] * W3[i*H:(i+1)*H] (host-precomputed).
  - out = leaky(sum_r sum_k P_k^r @ V_k + 2*b3)

Sharding: nodes split 8 ways (6250/core, padded to 6272 = 49*128).
The all-gathered feature table packs TWO nodes per 256-byte row
([25088, 128] bf16): local node position p -> row 64*(p//128) +
(p%128)//2, column-half p%2. int16 gather indices cover the whole
table; edges split by the src's column-half (== src position parity
== src node-id parity, enforced by the balance permutation giving
each block 32 even-id + 32 odd-id slots).

Aggregation processes 128-node SUPERBLOCK pairs: gathered subtiles of
the two 64-node blocks are interleaved [A_s|B_s] so each [128,128]
stationary load feeds one [128,128] one-hot matmul producing both
blocks' aggregates in opposite PSUM quadrants (2 subtiles per
LoadStationary).

Gather calls are one (pair, half) = 1024 idxs each: the SWDGE ring
holds 128 descriptors per SDMA engine, so 1024-idx calls (64/engine
+ sem) are the largest that decode without deadlock; the Q7
descriptor-push rate (~8.6us/call, 4 queues in parallel) is the hard
floor for the prop phases, which run the gpsimd engine at ~100%.

Overlap structure: MLP leaky-ReLUs run on the otherwise-idle ACT
engine; write_scaled is PE-transpose + one DVE broadcast-scale per
superblock, with 4 superblocks batched into ONE 3D DMA (DRAM addr =
64p + 8192j + f is affine in partition p). Each AllGather is split
into two chunk-aligned halves (table layout [8 x rows 0:1536 | 8 x
rows 1536:3200]) so the first half launches while the second is
still being produced, and mid-kernel AG triggers are emitted after
group 2 of the following prop so their input-waits never stall the
gather queue head. Bulk constant loads (idx/dstoff/dinv) are
deferred until after AG(0,0) is triggered; the final output stage is
fused per-pair into the last prop.
"""

import math
import os
import sys

import ml_dtypes
import numpy as np

for _p in ("/opt/trn_rl_repo", "/root/.axon_site/_ro/trn_rl_repo"):
    if os.path.isdir(_p) and _p not in sys.path:
        sys.path.insert(0, _p)

import concourse.bacc as bacc
import concourse.bass as bass
import concourse.mybir as mybir
import concourse.tile as tile
from concourse.bass_utils import run_bass_kernel_spmd

F32 = mybir.dt.float32
BF16 = mybir.dt.bfloat16
I16 = mybir.dt.int16


class Cfg:
    def __init__(self, N=50000, F=128, H=64, NCORES=8):
        self.N = N
        self.F = F
        self.H = H
        self.NCORES = NCORES
        self.NPC = N // NCORES              # real nodes per core
        self.BLK = 64                       # dst nodes per block
        self.NPAD = 6400                    # padded nodes per core (50*128)
        assert self.NPAD % 128 == 0 and self.NPAD >= self.NPC
        self.NBLK = self.NPAD // self.BLK   # 100
        self.NPAIR = self.NBLK // 2         # 50 superblocks of 128 nodes
        self.TPC = self.NPAD // 2           # table rows per core (3200)
        self.TROWS = self.TPC * NCORES      # 25600 (< 32768: int16 idx ok)
        self.GP = 1                         # pairs per gather call


def _calc_thetas(d=2):
    thetas = []
    for i in range(d + 1):
        p1 = np.zeros(i + 1)
        p1[i] = 0.5 ** i
        m = d - i
        p2 = np.array([math.comb(m, j) * (-0.5) ** j for j in range(m + 1)])
        c = np.convolve(p1, p2)
        beta = math.factorial(i) * math.factorial(d - i) / math.factorial(d + 1)
        thetas.append(c / beta)
    return np.stack(thetas)  # [3, 3] increasing power


THETAS = _calc_thetas(2)


def _pack_idx16(flat):
    """Q7 layout: idx i -> partition i%16, free i//16; replicated x8 groups."""
    n = len(flat)
    arr = flat.astype(np.int16).reshape(n // 16, 16).T  # [16, n/16]
    return np.tile(arr, (8, 1))  # [128, n/16]


def _greedy_balance(cnt4, parity, cfg, hard=False):
    """Assign nodes to blocks minimizing max per-(rel, src-col-half) count.

    Soft pass: slot parity prefers the node's parity, overflows flip.
    Hard pass (parity = previous pass's realized col-half): slot parity
    is fixed, so cnt4 (keyed on that col-half) is exact.
    """
    c = cfg
    order = np.argsort(-cnt4.sum(1), kind="stable")
    blk_cnt = np.zeros((c.NBLK, 4), np.int64)
    blk_n = np.zeros((c.NBLK, 2), np.int64)
    pos = np.zeros(len(cnt4), np.int64)
    BIG = 1 << 40
    for n in order:
        pi = int(parity[n])
        score = (blk_cnt + cnt4[n]).max(axis=1) + blk_cnt.max(axis=1)
        if hard:
            score[blk_n[:, pi] >= c.BLK // 2] = BIG
        else:
            score[blk_n.sum(1) >= c.BLK] = BIG
        b = int(np.argmin(score))
        if not hard and blk_n[b, pi] >= c.BLK // 2:
            pi = 1 - pi
        pos[n] = b * c.BLK + 2 * blk_n[b, pi] + pi
        blk_n[b, pi] += 1
        blk_cnt[b] += cnt4[n]
    return pos


def _prep_relation(src, dst, cfg, POS, SROW, SH):
    """Group one relation's edges by (dst-owner core, pair, src-half, block)."""
    c = cfg
    deg = np.bincount(dst, minlength=c.N).astype(np.float32)
    dinv = np.clip(deg, 1.0, None) ** -0.5
    per_core = []
    lh = 0
    for core in range(c.NCORES):
        m = (dst // c.NPC) == core
        s_row = SROW[src[m]]
        s_h = SH[src[m]]
        d_loc = POS[dst[m]] - core * c.NPAD
        pair = d_loc // 128
        ab = (d_loc % 128) // c.BLK
        off = (d_loc % 128).astype(np.float32)
        key = (pair * 2 + s_h) * 2 + ab
        order = np.lexsort((s_row, key))
        key_s, row_s, off_s = key[order], s_row[order], off[order]
        counts = np.bincount(key_s, minlength=c.NPAIR * 4)
        starts = np.concatenate(([0], np.cumsum(counts)[:-1]))
        within = np.arange(len(key_s)) - starts[key_s]
        lh = max(lh, int(counts.max()) if len(counts) else 0)
        per_core.append((key_s, within, row_s, off_s))
    return per_core, dinv, lh


def _finalize_relation(slots_pc, cfg, NS, skip_calls=0):
    """Build per-core packed idx + dstoff tensors for one relation.

    Trailing pad positions in each call's idx stream are set to -1: the
    Q7 ucode truncates trailing negatives before descriptor gen (the
    gathered SBUF stays stale, but pad one-hot columns are 0 so stale
    values never reach the aggregate). The first `skip_calls` calls keep
    real (spread junk-row) pads so every GTS slot's first write covers
    the full tile — stale uninitialized SBUF could be NaN, and NaN*0
    poisons the matmul.
    """
    c = cfg
    L = NS * 128
    out = []
    for core in range(c.NCORES):
        key_s, within, row_s, off_s = slots_pc[core]
        # pad slots get spread (distinct) rows: duplicate same-address
        # descriptors serialize the SDMA engines (~12x slower packets)
        idx_slots = np.broadcast_to(
            (np.arange(L) * 97 + 13) % c.TROWS, (c.NPAIR * 4, L)).copy()
        off_slots = np.full((c.NPAIR * 4, L), 255.0, np.float32)
        real_slots = np.zeros((c.NPAIR * 4, L), bool)
        idx_slots[key_s, within] = row_s
        off_slots[key_s, within] = off_s
        real_slots[key_s, within] = True
        # interleaved stream per (pair, h): subtile t = 2*s + ab
        sl = idx_slots.reshape(c.NPAIR, 2, 2, NS, 128)
        inter = sl.transpose(0, 1, 3, 2, 4)  # [q, h, s, ab, 128]
        rl = real_slots.reshape(c.NPAIR, 2, 2, NS, 128)
        rinter = rl.transpose(0, 1, 3, 2, 4)
        parts = []
        ci = 0
        for g in range(c.NPAIR // c.GP):
            for h in range(2):
                for q in range(g * c.GP, (g + 1) * c.GP):
                    stream = inter[q, h].reshape(-1).copy()
                    if ci >= skip_calls:
                        rm = rinter[q, h].reshape(-1)
                        nz = np.nonzero(rm)[0]
                        last = nz[-1] if len(nz) else -1
                        stream[last + 1 :] = -1
                    parts.append(_pack_idx16(stream))
                ci += 1
        idx_packed = np.concatenate(parts, axis=1)
        # dstoff col ((q*2+h)*2+ab)*NS + s = offs of subtile rows [128]
        do = (off_slots.reshape(c.NPAIR * 4, NS, 128)
              .transpose(2, 0, 1)
              .reshape(128, c.NPAIR * 4 * NS)
              .astype(ml_dtypes.bfloat16))
        out.append((idx_packed, do))
    return out


def _build(cfg, NS, debug=False):
    """Build the SPMD Bass graph (identical on all cores)."""
    c = cfg
    H = c.H
    NQ = 4
    NSLOT = 12
    GP = c.GP                             # pairs per gather call
    SUBS = GP * 2 * NS                    # subtiles per gather call

    nc = bacc.Bacc("TRN2", target_bir_lowering=False, debug=False,
                   num_devices=c.NCORES, num_swdge_queues=NQ,
                   dynamic_dma_scratch_size=49152)

    dram_in = {}

    def din(name, shape, dtype=F32):
        dram_in[name] = nc.dram_tensor(name, list(shape), dtype,
                                       kind="ExternalInput")
        return dram_in[name]

    IDXCOLS = 2 * c.NPAIR * 2 * NS * 128 // 16
    for r in range(2):
        din(f"featT{r}", (c.F, c.NPAD), BF16)
        din(f"idx{r}", (128, IDXCOLS), I16)
        din(f"dstoff{r}", (128, c.NPAIR * 4 * NS), BF16)
        din(f"W1_{r}", (c.F, H), BF16)
        din(f"W2_{r}", (H, H), BF16)
        din(f"b1_{r}", (H, 1))
        din(f"b2_{r}", (H, 1))
    din("dinv0", (H, c.NPAD), BF16)
    din("dinv1", (H, c.NPAD), BF16)
    din("dinvC0", (128, c.NPAIR))
    din("dinvC1", (128, c.NPAIR))
    din("Vk", (H, 3 * H), BF16)
    din("identb", (128, 128), BF16)
    din("b3bc", (128, H))
    din("iota", (128, 128), BF16)

    out_t = nc.dram_tensor("out", [c.NPAD, H], F32, kind="ExternalOutput")
    if debug:
        dbg = {
            "dbg_P0T": nc.dram_tensor("dbg_P0T", [H, c.NPAD], BF16,
                                      kind="ExternalOutput"),
            "dbg_tab": nc.dram_tensor("dbg_tab", [c.TROWS, 128], BF16,
                                      kind="ExternalOutput"),
            "dbg_P1T": nc.dram_tensor("dbg_P1T", [H, c.NPAD], BF16,
                                      kind="ExternalOutput"),
            "dbg_gt0": nc.dram_tensor("dbg_gt0", [128, 2 * NS * H], BF16,
                                      kind="ExternalOutput"),
            "dbg_gt1": nc.dram_tensor("dbg_gt1", [128, 2 * NS * H], BF16,
                                      kind="ExternalOutput"),
            "dbg_oh": nc.dram_tensor("dbg_oh", [128, 2 * NS * 128], BF16,
                                     kind="ExternalOutput"),
        }
        DBGQ = 4

    rg = [list(range(c.NCORES))]

    # per-call idx column base (in int16 columns) for call (g, h)
    CALL16 = GP * 2 * NS * 128 // 16
    call_base = {(g, h): (g * 2 + h) * CALL16
                 for g in range(c.NPAIR // GP) for h in range(2)}
    assert 2 * (c.NPAIR // GP) * CALL16 == IDXCOLS

    with tile.TileContext(nc) as tc:
        with (
            tc.tile_pool(name="const", bufs=1) as constp,
            tc.tile_pool(name="dram", bufs=1, space="DRAM") as dramp,
            tc.tile_pool(name="feat", bufs=6) as featp,
            tc.tile_pool(name="h1", bufs=4) as h1p,
            tc.tile_pool(name="oh", bufs=3) as ohp,
            tc.tile_pool(name="sc", bufs=4) as scp,
            tc.tile_pool(name="stg", bufs=5) as stgp,
            tc.tile_pool(name="psmlp", bufs=4, space="PSUM") as psmlp,
            tc.tile_pool(name="psagg", bufs=2, space="PSUM") as psagg,
            tc.tile_pool(name="psmisc", bufs=2, space="PSUM") as psmisc,
        ):
            def load_const(name, shape, dtype=F32, eng=None):
                t = constp.tile(list(shape), dtype, name=f"c_{name}")
                (eng or nc.scalar).dma_start(out=t[:], in_=dram_in[name].ap()[:])
                return t

            # essentials for mlp(0)+write_scaled (loaded up front)
            W1 = [load_const(f"W1_{r}", (c.F, H), BF16) for r in range(2)]
            W2 = [load_const(f"W2_{r}", (H, H), BF16) for r in range(2)]
            b1 = [load_const(f"b1_{r}", (H, 1)) for r in range(2)]
            b2 = [load_const(f"b2_{r}", (H, 1)) for r in range(2)]
            dinvC = [load_const(f"dinvC{r}", (128, c.NPAIR)) for r in range(2)]
            identb = load_const("identb", (128, 128), BF16)
            # bulk tensors only needed from the first prop (~150us in) are
            # loaded via deferred_loads() after AG(0,0) is triggered so the
            # DMA queues stay clear for featT/agin during startup.
            dinv = [None, None]
            dstoff = [None, None]
            idxsb = [None, None]
            Vk = b3bc = iota = None

            def deferred_loads():
                nonlocal Vk, b3bc, iota
                for r in range(2):
                    idxsb[r] = load_const(f"idx{r}", (128, IDXCOLS), I16)
                    dinv[r] = load_const(f"dinv{r}", (H, c.NPAD), BF16,
                                         eng=nc.sync)
                    dstoff[r] = load_const(f"dstoff{r}",
                                           (128, c.NPAIR * 4 * NS), BF16,
                                           eng=nc.sync)
                Vk = load_const("Vk", (H, 3 * H), BF16, eng=nc.sync)
                b3bc = load_const("b3bc", (128, H), eng=nc.sync)
                iota = load_const("iota", (128, 128), BF16, eng=nc.sync)

            # ---- persistent SBUF state ----
            P0T = [constp.tile([H, c.NPAD], BF16, name=f"P0T{r}") for r in range(2)]
            P1T = [constp.tile([H, c.NPAD], BF16, name=f"P1T{r}") for r in range(2)]
            outaccT = constp.tile([H, c.NPAD], BF16, name="outaccT")
            GTS = [constp.tile([128, SUBS, H], BF16, name=f"gtslot{i}")
                   for i in range(NSLOT)]

            # ---- internal DRAM ----
            agin = [[dramp.tile([c.TPC, 128], BF16, name=f"agin{r}_{hp}")
                     for hp in range(2)] for r in range(2)]
            table = [[dramp.tile([c.TROWS, 128], BF16, name=f"table{r}_{hp}")
                      for hp in range(2)] for r in range(2)]

            CHUNKS = []
            pos2 = 0
            while pos2 < c.NPAD:
                w = min(512, c.NPAD - pos2)
                CHUNKS.append((pos2, w))
                pos2 += w

            def mlp(r):
                for (p0, w) in CHUNKS:
                    ft = featp.tile([c.F, 512], BF16, name="ft")
                    nc.sync.dma_start(out=ft[:, :w],
                                      in_=dram_in[f"featT{r}"].ap()[:, p0 : p0 + w])
                    ps1 = psmlp.tile([H, 512], F32, name="ps1", tag="mlp")
                    nc.tensor.matmul(ps1[:, :w], W1[r][:], ft[:, :w],
                                     start=True, stop=True)
                    h1t = h1p.tile([H, 512], BF16, name="h1t")
                    nc.scalar.activation(h1t[:, :w], ps1[:, :w],
                                         mybir.ActivationFunctionType.Lrelu,
                                         bias=b1[r][:], alpha=0.01)
                    ps2 = psmlp.tile([H, 512], F32, name="ps2", tag="mlp")
                    nc.tensor.matmul(ps2[:, :w], W2[r][:], h1t[:, :w],
                                     start=True, stop=True)
                    nc.scalar.activation(P0T[r][:, p0 : p0 + w], ps2[:, :w],
                                         mybir.ActivationFunctionType.Lrelu,
                                         bias=b2[r][:], alpha=0.01)

            def write_scaled(PT, r, agin_t):
                """agin row 64*(p//128)+(p%128)//2, col-half p%2 <- dinv*PT.

                DRAM address is affine in partition: addr_el = 64p + 8192j + f
                (j = superblock within chunk), so a chunk's 2-4 superblocks
                go out in ONE 3D DMA instead of 4 small ones.
                """
                for (p0, w) in CHUNKS:
                    ns = w // 128
                    stg4 = stgp.tile([128, 4, H], BF16, name="stg4")
                    for si in range(ns):
                        s = p0 // 128 + si
                        tp = psmisc.tile([128, H], BF16, name="tp", tag="misc")
                        nc.tensor.transpose(
                            tp[:], PT[:, s * 128 : s * 128 + 128], identb[:H, :H])
                        nc.vector.tensor_tensor(
                            out=stg4[:, si, :], in0=tp[:],
                            in1=dinvC[r][:, s : s + 1].to_broadcast((128, H)),
                            op=mybir.AluOpType.mult)
                    out_ap = bass.AP(
                        tensor=agin_t[:].tensor, offset=p0 * 64,
                        ap=[[64, 128], [8192, ns], [1, H]])
                    nc.sync.dma_start(out=out_ap, in_=stg4[:, 0:ns, :])

            def allgather(r, hp):
                ofs = 0
                for (st, ln) in ((0, 1536), (1536, c.TPC - 1536)):
                    nc.gpsimd.collective_compute(
                        "AllGather",
                        mybir.AluOpType.bypass,
                        replica_groups=rg,
                        ins=[agin[r][hp][st : st + ln, :].opt()],
                        outs=[table[r][hp][ofs : ofs + c.NCORES * ln, :].opt()],
                    )
                    ofs += c.NCORES * ln

            def prop(r, hop, after2=None):
                """hop=1: P1T = L~ P0T. hop=2: fused output accumulation.

                after2() is emitted after group 2's calls so a following
                collective trigger doesn't block the gather queue head
                while its input is still being written."""
                tab = table[r][hop - 1]
                PTin = P0T[r] if hop == 1 else P1T[r]

                def issue_call(g, h):
                    b16 = call_base[(g, h)]
                    gt = GTS[(g * 2 + h) % NSLOT]
                    src_ap = tab[:, 0:H] if h == 0 else tab[:, H : 2 * H]
                    n_i = GP * 2 * NS * 128
                    _dma_gather_narrow(
                        nc.gpsimd, gt[:, 0:SUBS, :], src_ap,
                        idxsb[r][:, b16 : b16 + n_i // 16],
                        n_i, n_i, H, 2 * H,
                        queue_num=(g * 2 + h) % NQ)
                    return gt

                def do_pair(q, gts):
                    bs = slice(q * 128, (q + 1) * 128)
                    oh = ohp.tile([128, 2 * NS, 128], BF16, name="oh")
                    for h in range(2):
                        for ab in range(2):
                            col = ((q * 2 + h) * 2 + ab) * NS
                            nc.vector.tensor_tensor(
                                out=oh[:, h * NS : (h + 1) * NS,
                                       ab * 64 : ab * 64 + 64],
                                in0=dstoff[r][:, col : col + NS][:, :, None]
                                    .to_broadcast((128, NS, 64)),
                                in1=iota[:][:, None, ab * 64 : ab * 64 + 64]
                                    .to_broadcast((128, NS, 64)),
                                op=mybir.AluOpType.is_equal,
                            )
                    if debug and r == 0 and hop == 1 and q == DBGQ:
                        nc.sync.dma_start(out=dbg["dbg_gt0"].ap()[:],
                                          in_=gts[0][:, :, :])
                        nc.sync.dma_start(out=dbg["dbg_gt1"].ap()[:],
                                          in_=gts[1][:, :, :])
                        nc.sync.dma_start(out=dbg["dbg_oh"].ap()[:],
                                          in_=oh[:, :, :])
                    ql = q % GP
                    agg = psagg.tile([128, 128], F32, name="agg")
                    k = 0
                    for h in range(2):
                        base = ql * 2 * NS
                        for s in range(NS):
                            nc.tensor.matmul(
                                agg[:],
                                gts[h][:, base + 2 * s : base + 2 * s + 2, :],
                                oh[:, h * NS + s, :],
                                start=(k == 0),
                                stop=(k == 2 * NS - 1),
                            )
                            k += 1
                    # tmp = dinv * agg (combine the two useful quadrants)
                    tmp = scp.tile([H, 128], BF16, name="tmp")
                    dv = dinv[r][:, bs]
                    nc.vector.tensor_tensor(out=tmp[:, 0:64], in0=agg[0:64, 0:64],
                                            in1=dv[:, 0:64],
                                            op=mybir.AluOpType.mult)
                    nc.vector.tensor_tensor(out=tmp[:, 64:128],
                                            in0=agg[64:128, 64:128],
                                            in1=dv[:, 64:128],
                                            op=mybir.AluOpType.mult)
                    if hop == 1:
                        nc.vector.tensor_tensor(out=P1T[r][:, bs],
                                                in0=PTin[:, bs], in1=tmp[:],
                                                op=mybir.AluOpType.subtract)
                    else:
                        p2 = scp.tile([H, 128], BF16, name="p2")
                        nc.vector.tensor_tensor(out=p2[:], in0=PTin[:, bs],
                                                in1=tmp[:],
                                                op=mybir.AluOpType.subtract)
                        op_ps = psmisc.tile([H, 128], F32, name="opps", tag="misc")
                        nc.tensor.matmul(op_ps[:], Vk[:, 0:H], P0T[r][:, bs],
                                         start=True, stop=False)
                        nc.tensor.matmul(op_ps[:], Vk[:, H : 2 * H],
                                         P1T[r][:, bs], start=False, stop=False)
                        nc.tensor.matmul(op_ps[:], Vk[:, 2 * H : 3 * H], p2[:],
                                         start=False, stop=True)
                        if r == 0:
                            nc.vector.tensor_copy(out=outaccT[:, bs],
                                                  in_=op_ps[:])
                        else:
                            nc.vector.tensor_add(out=outaccT[:, bs],
                                                 in0=outaccT[:, bs],
                                                 in1=op_ps[:])
                            final_pair(q)

                LOOK = 4
                NG = c.NPAIR // GP
                gts_by_g = {}
                for g in range(min(LOOK + 1, NG)):
                    gts_by_g[g] = [issue_call(g, 0), issue_call(g, 1)]
                for g in range(NG):
                    g2 = g + LOOK + 1
                    if g2 < NG:
                        gts_by_g[g2] = [issue_call(g2, 0), issue_call(g2, 1)]
                    if g == 2 and after2 is not None:
                        after2()
                    for q in range(g * GP, (g + 1) * GP):
                        do_pair(q, gts_by_g[g])
                    gts_by_g.pop(g)

            def final_pair(q):
                bs = slice(q * 128, (q + 1) * 128)
                tp = psmisc.tile([128, H], BF16, name="tpo", tag="misc")
                nc.tensor.transpose(tp[:], outaccT[:, bs], identb[:H, :H])
                t = scp.tile([128, H], F32, name="tf")
                nc.vector.tensor_tensor(out=t[:], in0=tp[:], in1=b3bc[:],
                                        op=mybir.AluOpType.add)
                so = stgp.tile([128, H], F32, name="so")
                nc.vector.scalar_tensor_tensor(
                    out=so[:], in0=t[:], scalar=0.01, in1=t[:],
                    op0=mybir.AluOpType.mult, op1=mybir.AluOpType.max)
                nc.sync.dma_start(out=out_t.ap()[bs, :], in_=so[:])

            mlp(0)
            write_scaled(P0T[0], 0, agin[0][0])
            allgather(0, 0)
            deferred_loads()
            if debug:
                nc.sync.dma_start(out=dbg["dbg_P0T"].ap()[:], in_=P0T[0][:])
                nc.sync.dma_start(out=dbg["dbg_tab"].ap()[:],
                                  in_=table[0][0][:])
            mlp(1)
            write_scaled(P0T[1], 1, agin[1][0])
            prop(0, 1, after2=lambda: allgather(1, 0))
            if debug:
                nc.sync.dma_start(out=dbg["dbg_P1T"].ap()[:], in_=P1T[0][:])
            write_scaled(P1T[0], 0, agin[0][1])
            prop(1, 1, after2=lambda: allgather(0, 1))
            write_scaled(P1T[1], 1, agin[1][1])
            prop(0, 2, after2=lambda: allgather(1, 1))
            prop(1, 2)

    nc.compile()
    return nc


def _prepare(inputs, cfg):
    c = cfg
    H = c.H
    W3 = inputs["W3"]
    V = np.zeros((H, 3 * H), np.float32)
    for k in range(3):
        acc = np.zeros((H, H), np.float64)
        for i in range(3):
            acc += THETAS[i][k] * W3[i * H : (i + 1) * H].astype(np.float64)
        V[:, k * H : (k + 1) * H] = acc.astype(np.float32)

    srcs = [np.asarray(inputs["src_r1"]).astype(np.int64),
            np.asarray(inputs["src_r2"]).astype(np.int64)]
    dsts = [np.asarray(inputs["dst_r1"]).astype(np.int64),
            np.asarray(inputs["dst_r2"]).astype(np.int64)]

    # Parity-constrained balance: node n only takes a position p with
    # p%2 == n%2, so a source's table column-half (p%2) is known up front.
    POS = np.zeros(c.N, np.int64)
    H1 = np.zeros(c.N, np.int64)
    for it in range(2):
        hsrc = [(srcs[r] % 2) if it == 0 else H1[srcs[r]] for r in range(2)]
        for core in range(c.NCORES):
            cnt4 = np.zeros((c.NPC, 4), np.int64)
            for r in range(2):
                m = (dsts[r] // c.NPC) == core
                d_loc = dsts[r][m] - core * c.NPC
                np.add.at(cnt4, (d_loc, 2 * r + hsrc[r][m]), 1)
            gids = np.arange(core * c.NPC, (core + 1) * c.NPC)
            pos = _greedy_balance(cnt4, (gids % 2) if it == 0 else H1[gids],
                                  c, hard=(it == 1))
            POS[gids] = core * c.NPAD + pos
        H1 = POS % 2

    # node global padded position -> table row / column-half.
    # Table layout (chunked AllGather): [8 x rows 0:1536 | 8 x rows 1536:TPC]
    loc = POS % c.NPAD
    core_of = POS // c.NPAD
    lr = (loc // 128) * c.BLK + (loc % 128) // 2
    SROW = np.where(lr < 1536, core_of * 1536 + lr,
                    c.NCORES * 1536 + core_of * (c.TPC - 1536) + (lr - 1536))
    SH = loc % 2

    rels = []
    LH = 128
    for r in range(2):
        slots_pc, dinv, lh = _prep_relation(srcs[r], dsts[r], c, POS, SROW, SH)
        rels.append((slots_pc, dinv))
        LH = max(LH, lh)
    NS = -(-LH // 128)

    # relation 0's prop runs first: its first 2*NSLOT(=24) calls must
    # fully initialize the GTS slots (see _finalize_relation docstring)
    fin = [_finalize_relation(rels[r][0], c, NS, skip_calls=24 if r == 0 else 0)
           for r in range(2)]

    iota = np.broadcast_to(np.arange(128, dtype=np.float32), (128, 128))
    ident = np.eye(128, dtype=np.float32)

    in_maps = []
    for core in range(c.NCORES):
        m = {}
        lpos_all = POS[core * c.NPC : (core + 1) * c.NPC] - core * c.NPAD
        dvT = np.zeros((2, H, c.NPAD), np.float32)
        for r in range(2):
            idx_packed, do = fin[r][core]
            m[f"idx{r}"] = idx_packed
            m[f"dstoff{r}"] = do
            feat = np.asarray(inputs["feat_r1" if r == 0 else "feat_r2"],
                              np.float32)
            ft = np.zeros((c.F, c.NPAD), np.float32)
            ft[:, lpos_all] = feat[core * c.NPC : (core + 1) * c.NPC].T
            m[f"featT{r}"] = ft.astype(ml_dtypes.bfloat16)
            dvT[r][:, lpos_all] = np.broadcast_to(
                rels[r][1][core * c.NPC : (core + 1) * c.NPC], (H, c.NPC))
            suf = "_r1" if r == 0 else "_r2"
            m[f"W1_{r}"] = np.asarray(inputs[f"W1{suf}"],
                                      np.float32).astype(ml_dtypes.bfloat16)
            m[f"W2_{r}"] = np.asarray(inputs[f"W2{suf}"],
                                      np.float32).astype(ml_dtypes.bfloat16)
            m[f"b1_{r}"] = np.asarray(inputs[f"b1{suf}"], np.float32).reshape(H, 1)
            m[f"b2_{r}"] = np.asarray(inputs[f"b2{suf}"], np.float32).reshape(H, 1)
        m["dinv0"] = dvT[0].astype(ml_dtypes.bfloat16)
        m["dinv1"] = dvT[1].astype(ml_dtypes.bfloat16)
        m["dinvC0"] = np.ascontiguousarray(dvT[0][0].reshape(c.NPAIR, 128).T)
        m["dinvC1"] = np.ascontiguousarray(dvT[1][0].reshape(c.NPAIR, 128).T)
        m["Vk"] = V.astype(ml_dtypes.bfloat16)
        m["b3bc"] = np.broadcast_to(
            2.0 * np.asarray(inputs["b3"], np.float32), (128, H)).copy()
        m["identb"] = ident.astype(ml_dtypes.bfloat16)
        m["iota"] = iota.astype(ml_dtypes.bfloat16).copy()
        in_maps.append(m)
    return in_maps, NS, POS


def _dma_gather_narrow(gp, out_ap, in_ap, idxs_ap, num_idxs, num_idxs_reg,
                       elem_size, elem_step, queue_num=0):
    """bass.BassGpSimd.dma_gather clone allowing elem_size_bytes % 256 != 0.

    The Q7 kernel (dma_gather.cpp gen_descs, non-transpose HBM path) supports
    any payload length; only the row STRIDE must encode as stride_bytes_256.
    Used to gather 128B bf16 rows from a 256B-strided table.
    """
    import concourse.ap_utils as ap_utils
    assert idxs_ap.dtype == I16
    assert in_ap.dtype == out_ap.dtype
    assert in_ap.space == bass.MemorySpace.DRAM
    assert idxs_ap.space == bass.MemorySpace.SBUF
    assert out_ap.space == bass.MemorySpace.SBUF
    assert ap_utils.ap_is_contiguous(out_ap.ap[1:])
    assert ap_utils.ap_is_contiguous(idxs_ap.ap[1:])
    assert in_ap.ap[0][0] == elem_step
    assert in_ap.ap[-1][1] == elem_size
    assert out_ap.ap[-1][1] == elem_size
    assert out_ap.ap[0][1] * out_ap.ap[1][1] * 1 >= num_idxs
    stride_bytes = elem_step * mybir.dt.size(in_ap.dtype)
    assert stride_bytes % 256 == 0 and stride_bytes // 256 < 256
    _in_ap = gp.lower_ap_dma(in_ap, for_custom_bir_dma=True)
    _idxs_ap = gp.lower_ap(idxs_ap)
    _out_ap = gp.lower_ap(out_ap)
    return gp.add_instruction(
        mybir.InstDMAGatherAnt(
            name=gp.bass.get_next_instruction_name(),
            ins=[*_in_ap, _idxs_ap, gp.lower_val_access(gp.to_reg(num_idxs_reg))],
            outs=[_out_ap],
            transpose=False,
            num_idxs=num_idxs,
            elem_size=elem_size,
            stride_bytes_256=stride_bytes // 256,
            gen_mode=0,
            single_packet=True,
            queue_num=queue_num,
            sbuf_tokens_per_rank=0,
            sbuf_free_dim_per_rank=0,
            sbuf_free_dim_pad_per_rank=0,
            sbuf_byte_offset=0,
        )
    )


_CACHE = {}


def _install_profile_shim():
    """Provide antenv.axon_hooks (missing in this image) so trace=True works."""
    try:
        from antenv.axon_hooks import get_axon_ntff_profile_hook  # noqa: F401
        return
    except ImportError:
        pass
    import types

    import antenv
    try:
        from trn_agent_boot.trn_boot import _ntff_profile_via_ctypes
        hook = _ntff_profile_via_ctypes("/opt/axon/libaxon_pjrt.so")
    except Exception:
        hook = None
    mod = types.ModuleType("antenv.axon_hooks")
    mod._hook = hook
    mod.get_axon_ntff_profile_hook = lambda: mod._hook

    def _set(h):
        mod._hook = h

    mod.set_axon_ntff_profile_hook = _set
    sys.modules["antenv.axon_hooks"] = mod
    antenv.axon_hooks = mod


def _run(inputs, trace=False, **kw):
    if trace:
        _install_profile_shim()
    cfg = Cfg(N=int(np.asarray(inputs["feat_r1"]).shape[0]))
    in_maps, NS, POS = _prepare(inputs, cfg)
    key = (cfg.N, NS, bool(kw.pop("debug", False)))
    if key not in _CACHE:
        _CACHE[key] = _build(cfg, NS, debug=key[2])
    nc = _CACHE[key]
    res = run_bass_kernel_spmd(nc, in_maps, core_ids=list(range(cfg.NCORES)),
                               trace=trace, **kw)
    outs = []
    for core in range(cfg.NCORES):
        lpos = POS[core * cfg.NPC : (core + 1) * cfg.NPC] - core * cfg.NPAD
        outs.append(np.asarray(res.results[core]["out"])[lpos])
    full = np.concatenate(outs, axis=0)
    return full, res


def kernel(**inputs):
    full, _ = _run(inputs, trace=False)
    return full



# revision 48
# speedup vs baseline: 1.0267x; 1.0081x over previous
"""BWGNN_Hetero Trainium2 kernel (8 NeuronCores, SPMD).

Math restructure of the reference:
  - poly_conv with THETAS (Bernstein, D=2) shares the Krylov basis
    P0 = h, P1 = L~ P0, P2 = L~ P1 across all three thetas, where
    L~ x = x - dinv * segsum((x*dinv)[src], dst).
  - concat(h_theta) @ W3 collapses to sum_k P_k @ V_k with
    V_k = sum_i THETA[i][k] * W3[i*H:(i+1)*H] (host-precomputed).
  - out = leaky(sum_r sum_k P_k^r @ V_k + 2*b3)

Sharding: nodes split 8 ways (6250/core, padded to 6272 = 49*128).
The all-gathered feature table packs TWO nodes per 256-byte row
([25088, 128] bf16): local node position p -> row 64*(p//128) +
(p%128)//2, column-half p%2. int16 gather indices cover the whole
table; edges split by the src's column-half (== src position parity
== src node-id parity, enforced by the balance permutation giving
each block 32 even-id + 32 odd-id slots).

Aggregation processes 128-node SUPERBLOCK pairs: gathered subtiles of
the two 64-node blocks are interleaved [A_s|B_s] so each [128,128]
stationary load feeds one [128,128] one-hot matmul producing both
blocks' aggregates in opposite PSUM quadrants (2 subtiles per
LoadStationary).

Gather calls are one (pair, half) = 1024 idxs each: the SWDGE ring
holds 128 descriptors per SDMA engine, so 1024-idx calls (64/engine
+ sem) are the largest that decode without deadlock; the Q7
descriptor-push rate (~8.6us/call, 4 queues in parallel) is the hard
floor for the prop phases, which run the gpsimd engine at ~100%.

Overlap structure: MLP leaky-ReLUs run on the otherwise-idle ACT
engine; write_scaled is PE-transpose + one DVE broadcast-scale per
superblock, with 4 superblocks batched into ONE 3D DMA (DRAM addr =
64p + 8192j + f is affine in partition p). Each AllGather is split
into two chunk-aligned halves (table layout [8 x rows 0:1536 | 8 x
rows 1536:3200]) so the first half launches while the second is
still being produced, and mid-kernel AG triggers are emitted after
group 2 of the following prop so their input-waits never stall the
gather queue head. Bulk constant loads (idx/dstoff/dinv) are
deferred until after AG(0,0) is triggered; the final output stage is
fused per-pair into the last prop.
"""

import math
import os
import sys

import ml_dtypes
import numpy as np

for _p in ("/opt/trn_rl_repo", "/root/.axon_site/_ro/trn_rl_repo"):
    if os.path.isdir(_p) and _p not in sys.path:
        sys.path.insert(0, _p)

import concourse.bacc as bacc
import concourse.bass as bass
import concourse.mybir as mybir
import concourse.tile as tile
from concourse.bass_utils import run_bass_kernel_spmd

F32 = mybir.dt.float32
BF16 = mybir.dt.bfloat16
I16 = mybir.dt.int16


class Cfg:
    def __init__(self, N=50000, F=128, H=64, NCORES=8):
        self.N = N
        self.F = F
        self.H = H
        self.NCORES = NCORES
        self.NPC = N // NCORES              # real nodes per core
        self.BLK = 64                       # dst nodes per block
        self.NPAD = 6400                    # padded nodes per core (50*128)
        assert self.NPAD % 128 == 0 and self.NPAD >= self.NPC
        self.NBLK = self.NPAD // self.BLK   # 100
        self.NPAIR = self.NBLK // 2         # 50 superblocks of 128 nodes
        self.TPC = self.NPAD // 2           # table rows per core (3200)
        self.TROWS = self.TPC * NCORES      # 25600 (< 32768: int16 idx ok)
        self.GP = 1                         # pairs per gather call


def _calc_thetas(d=2):
    thetas = []
    for i in range(d + 1):
        p1 = np.zeros(i + 1)
        p1[i] = 0.5 ** i
        m = d - i
        p2 = np.array([math.comb(m, j) * (-0.5) ** j for j in range(m + 1)])
        c = np.convolve(p1, p2)
        beta = math.factorial(i) * math.factorial(d - i) / math.factorial(d + 1)
        thetas.append(c / beta)
    return np.stack(thetas)  # [3, 3] increasing power


THETAS = _calc_thetas(2)


def _pack_idx16(flat):
    """Q7 layout: idx i -> partition i%16, free i//16; replicated x8 groups."""
    n = len(flat)
    arr = flat.astype(np.int16).reshape(n // 16, 16).T  # [16, n/16]
    return np.tile(arr, (8, 1))  # [128, n/16]


def _greedy_balance(cnt4, parity, cfg, hard=False):
    """Assign nodes to blocks minimizing max per-(rel, src-col-half) count.

    Soft pass: slot parity prefers the node's parity, overflows flip.
    Hard pass (parity = previous pass's realized col-half): slot parity
    is fixed, so cnt4 (keyed on that col-half) is exact.
    """
    c = cfg
    order = np.argsort(-cnt4.sum(1), kind="stable")
    blk_cnt = np.zeros((c.NBLK, 4), np.int64)
    blk_n = np.zeros((c.NBLK, 2), np.int64)
    pos = np.zeros(len(cnt4), np.int64)
    BIG = 1 << 40
    for n in order:
        pi = int(parity[n])
        score = (blk_cnt + cnt4[n]).max(axis=1) + blk_cnt.max(axis=1)
        if hard:
            score[blk_n[:, pi] >= c.BLK // 2] = BIG
        else:
            score[blk_n.sum(1) >= c.BLK] = BIG
        b = int(np.argmin(score))
        if not hard and blk_n[b, pi] >= c.BLK // 2:
            pi = 1 - pi
        pos[n] = b * c.BLK + 2 * blk_n[b, pi] + pi
        blk_n[b, pi] += 1
        blk_cnt[b] += cnt4[n]
    return pos


def _prep_relation(src, dst, cfg, POS, SROW, SH):
    """Group one relation's edges by (dst-owner core, pair, src-half, block)."""
    c = cfg
    deg = np.bincount(dst, minlength=c.N).astype(np.float32)
    dinv = np.clip(deg, 1.0, None) ** -0.5
    per_core = []
    lh = 0
    for core in range(c.NCORES):
        m = (dst // c.NPC) == core
        s_row = SROW[src[m]]
        s_h = SH[src[m]]
        d_loc = POS[dst[m]] - core * c.NPAD
        pair = d_loc // 128
        ab = (d_loc % 128) // c.BLK
        off = (d_loc % 128).astype(np.float32)
        key = (pair * 2 + s_h) * 2 + ab
        order = np.lexsort((s_row, key))
        key_s, row_s, off_s = key[order], s_row[order], off[order]
        counts = np.bincount(key_s, minlength=c.NPAIR * 4)
        starts = np.concatenate(([0], np.cumsum(counts)[:-1]))
        within = np.arange(len(key_s)) - starts[key_s]
        lh = max(lh, int(counts.max()) if len(counts) else 0)
        per_core.append((key_s, within, row_s, off_s))
    return per_core, dinv, lh


def _finalize_relation(slots_pc, cfg, NS, skip_calls=0):
    """Build per-core packed idx + dstoff tensors for one relation.

    Trailing pad positions in each call's idx stream are set to -1: the
    Q7 ucode truncates trailing negatives before descriptor gen (the
    gathered SBUF stays stale, but pad one-hot columns are 0 so stale
    values never reach the aggregate). The first `skip_calls` calls keep
    real (spread junk-row) pads so every GTS slot's first write covers
    the full tile — stale uninitialized SBUF could be NaN, and NaN*0
    poisons the matmul.
    """
    c = cfg
    L = NS * 128
    out = []
    for core in range(c.NCORES):
        key_s, within, row_s, off_s = slots_pc[core]
        # pad slots get spread (distinct) rows: duplicate same-address
        # descriptors serialize the SDMA engines (~12x slower packets)
        idx_slots = np.broadcast_to(
            (np.arange(L) * 97 + 13) % c.TROWS, (c.NPAIR * 4, L)).copy()
        off_slots = np.full((c.NPAIR * 4, L), 255.0, np.float32)
        real_slots = np.zeros((c.NPAIR * 4, L), bool)
        idx_slots[key_s, within] = row_s
        off_slots[key_s, within] = off_s
        real_slots[key_s, within] = True
        # interleaved stream per (pair, h): subtile t = 2*s + ab
        sl = idx_slots.reshape(c.NPAIR, 2, 2, NS, 128)
        inter = sl.transpose(0, 1, 3, 2, 4)  # [q, h, s, ab, 128]
        rl = real_slots.reshape(c.NPAIR, 2, 2, NS, 128)
        rinter = rl.transpose(0, 1, 3, 2, 4)
        parts = []
        ci = 0
        for g in range(c.NPAIR // c.GP):
            for h in range(2):
                for q in range(g * c.GP, (g + 1) * c.GP):
                    stream = inter[q, h].reshape(-1).copy()
                    if ci >= skip_calls:
                        rm = rinter[q, h].reshape(-1)
                        nz = np.nonzero(rm)[0]
                        last = nz[-1] if len(nz) else -1
                        stream[last + 1 :] = -1
                    parts.append(_pack_idx16(stream))
                ci += 1
        idx_packed = np.concatenate(parts, axis=1)
        # dstoff col ((q*2+h)*2+ab)*NS + s = offs of subtile rows [128]
        do = (off_slots.reshape(c.NPAIR * 4, NS, 128)
              .transpose(2, 0, 1)
              .reshape(128, c.NPAIR * 4 * NS)
              .astype(ml_dtypes.bfloat16))
        out.append((idx_packed, do))
    return out


def _build(cfg, NS, debug=False):
    """Build the SPMD Bass graph (identical on all cores)."""
    c = cfg
    H = c.H
    NQ = 4
    NSLOT = 12
    GP = c.GP                             # pairs per gather call
    SUBS = GP * 2 * NS                    # subtiles per gather call

    nc = bacc.Bacc("TRN2", target_bir_lowering=False, debug=False,
                   num_devices=c.NCORES, num_swdge_queues=NQ,
                   dynamic_dma_scratch_size=49152)

    dram_in = {}

    def din(name, shape, dtype=F32):
        dram_in[name] = nc.dram_tensor(name, list(shape), dtype,
                                       kind="ExternalInput")
        return dram_in[name]

    IDXCOLS = 2 * c.NPAIR * 2 * NS * 128 // 16
    for r in range(2):
        din(f"featT{r}", (c.F, c.NPAD), BF16)
        din(f"idx{r}", (128, IDXCOLS), I16)
        din(f"dstoff{r}", (128, c.NPAIR * 4 * NS), BF16)
        din(f"W1_{r}", (c.F, H), BF16)
        din(f"W2_{r}", (H, H), BF16)
        din(f"b1_{r}", (H, 1))
        din(f"b2_{r}", (H, 1))
    din("dinv0", (H, c.NPAD), BF16)
    din("dinv1", (H, c.NPAD), BF16)
    din("dinvC0", (128, c.NPAIR))
    din("dinvC1", (128, c.NPAIR))
    din("Vk", (H, 3 * H), BF16)
    din("identb", (128, 128), BF16)
    din("b3bc", (128, H))
    din("iota", (128, 128), BF16)

    out_t = nc.dram_tensor("out", [c.NPAD, H], F32, kind="ExternalOutput")
    if debug:
        dbg = {
            "dbg_P0T": nc.dram_tensor("dbg_P0T", [H, c.NPAD], BF16,
                                      kind="ExternalOutput"),
            "dbg_tab": nc.dram_tensor("dbg_tab", [c.TROWS, 128], BF16,
                                      kind="ExternalOutput"),
            "dbg_P1T": nc.dram_tensor("dbg_P1T", [H, c.NPAD], BF16,
                                      kind="ExternalOutput"),
            "dbg_gt0": nc.dram_tensor("dbg_gt0", [128, 2 * NS * H], BF16,
                                      kind="ExternalOutput"),
            "dbg_gt1": nc.dram_tensor("dbg_gt1", [128, 2 * NS * H], BF16,
                                      kind="ExternalOutput"),
            "dbg_oh": nc.dram_tensor("dbg_oh", [128, 2 * NS * 128], BF16,
                                     kind="ExternalOutput"),
        }
        DBGQ = 4

    rg = [list(range(c.NCORES))]

    # per-call idx column base (in int16 columns) for call (g, h)
    CALL16 = GP * 2 * NS * 128 // 16
    call_base = {(g, h): (g * 2 + h) * CALL16
                 for g in range(c.NPAIR // GP) for h in range(2)}
    assert 2 * (c.NPAIR // GP) * CALL16 == IDXCOLS

    with tile.TileContext(nc) as tc:
        with (
            tc.tile_pool(name="const", bufs=1) as constp,
            tc.tile_pool(name="dram", bufs=1, space="DRAM") as dramp,
            tc.tile_pool(name="feat", bufs=6) as featp,
            tc.tile_pool(name="h1", bufs=4) as h1p,
            tc.tile_pool(name="oh", bufs=3) as ohp,
            tc.tile_pool(name="sc", bufs=4) as scp,
            tc.tile_pool(name="stg", bufs=5) as stgp,
            tc.tile_pool(name="psmlp", bufs=3, space="PSUM") as psmlp,
            tc.tile_pool(name="psagg", bufs=2, space="PSUM") as psagg,
            tc.tile_pool(name="psmisc", bufs=3, space="PSUM") as psmisc,
        ):
            def load_const(name, shape, dtype=F32, eng=None):
                t = constp.tile(list(shape), dtype, name=f"c_{name}")
                (eng or nc.scalar).dma_start(out=t[:], in_=dram_in[name].ap()[:])
                return t

            # essentials for mlp(0)+write_scaled (loaded up front)
            W1 = [load_const(f"W1_{r}", (c.F, H), BF16) for r in range(2)]
            W2 = [load_const(f"W2_{r}", (H, H), BF16) for r in range(2)]
            b1 = [load_const(f"b1_{r}", (H, 1)) for r in range(2)]
            b2 = [load_const(f"b2_{r}", (H, 1)) for r in range(2)]
            dinvC = [load_const(f"dinvC{r}", (128, c.NPAIR)) for r in range(2)]
            identb = load_const("identb", (128, 128), BF16)
            # bulk tensors only needed from the first prop (~150us in) are
            # loaded via deferred_loads() after AG(0,0) is triggered so the
            # DMA queues stay clear for featT/agin during startup.
            dinv = [None, None]
            dstoff = [None, None]
            idxsb = [None, None]
            Vk = b3bc = iota = None

            def deferred_loads():
                nonlocal Vk, b3bc, iota
                for r in range(2):
                    idxsb[r] = load_const(f"idx{r}", (128, IDXCOLS), I16)
                    dinv[r] = load_const(f"dinv{r}", (H, c.NPAD), BF16,
                                         eng=nc.sync)
                    dstoff[r] = load_const(f"dstoff{r}",
                                           (128, c.NPAIR * 4 * NS), BF16,
                                           eng=nc.sync)
                Vk = load_const("Vk", (H, 3 * H), BF16, eng=nc.sync)
                b3bc = load_const("b3bc", (128, H), eng=nc.sync)
                iota = load_const("iota", (128, 128), BF16, eng=nc.sync)

            # ---- persistent SBUF state ----
            P0T = [constp.tile([H, c.NPAD], BF16, name=f"P0T{r}") for r in range(2)]
            P1T = [constp.tile([H, c.NPAD], BF16, name=f"P1T{r}") for r in range(2)]
            outaccT = constp.tile([H, c.NPAD], BF16, name="outaccT")
            GTS = [constp.tile([128, SUBS, H], BF16, name=f"gtslot{i}")
                   for i in range(NSLOT)]

            # ---- internal DRAM ----
            agin = [[dramp.tile([c.TPC, 128], BF16, name=f"agin{r}_{hp}")
                     for hp in range(2)] for r in range(2)]
            table = [[dramp.tile([c.TROWS, 128], BF16, name=f"table{r}_{hp}")
                      for hp in range(2)] for r in range(2)]

            CHUNKS = []
            pos2 = 0
            while pos2 < c.NPAD:
                w = min(512, c.NPAD - pos2)
                CHUNKS.append((pos2, w))
                pos2 += w

            def mlp(r):
                for (p0, w) in CHUNKS:
                    ft = featp.tile([c.F, 512], BF16, name="ft")
                    nc.sync.dma_start(out=ft[:, :w],
                                      in_=dram_in[f"featT{r}"].ap()[:, p0 : p0 + w])
                    ps1 = psmlp.tile([H, 512], F32, name="ps1", tag="mlp")
                    nc.tensor.matmul(ps1[:, :w], W1[r][:], ft[:, :w],
                                     start=True, stop=True)
                    h1t = h1p.tile([H, 512], BF16, name="h1t")
                    nc.scalar.activation(h1t[:, :w], ps1[:, :w],
                                         mybir.ActivationFunctionType.Lrelu,
                                         bias=b1[r][:], alpha=0.01)
                    ps2 = psmlp.tile([H, 512], F32, name="ps2", tag="mlp")
                    nc.tensor.matmul(ps2[:, :w], W2[r][:], h1t[:, :w],
                                     start=True, stop=True)
                    nc.scalar.activation(P0T[r][:, p0 : p0 + w], ps2[:, :w],
                                         mybir.ActivationFunctionType.Lrelu,
                                         bias=b2[r][:], alpha=0.01)

            def write_scaled(PT, r, agin_t):
                """agin row 64*(p//128)+(p%128)//2, col-half p%2 <- dinv*PT.

                DRAM address is affine in partition: addr_el = 64p + 8192j + f
                (j = superblock within chunk), so a chunk's 2-4 superblocks
                go out in ONE 3D DMA instead of 4 small ones.
                """
                for (p0, w) in CHUNKS:
                    ns = w // 128
                    stg4 = stgp.tile([128, 4, H], BF16, name="stg4")
                    for si in range(ns):
                        s = p0 // 128 + si
                        tp = psmisc.tile([128, H], BF16, name="tp", tag="misc")
                        nc.tensor.transpose(
                            tp[:], PT[:, s * 128 : s * 128 + 128], identb[:H, :H])
                        nc.vector.tensor_tensor(
                            out=stg4[:, si, :], in0=tp[:],
                            in1=dinvC[r][:, s : s + 1].to_broadcast((128, H)),
                            op=mybir.AluOpType.mult)
                    out_ap = bass.AP(
                        tensor=agin_t[:].tensor, offset=p0 * 64,
                        ap=[[64, 128], [8192, ns], [1, H]])
                    nc.sync.dma_start(out=out_ap, in_=stg4[:, 0:ns, :])

            def allgather(r, hp):
                ofs = 0
                for (st, ln) in ((0, 1536), (1536, c.TPC - 1536)):
                    nc.gpsimd.collective_compute(
                        "AllGather",
                        mybir.AluOpType.bypass,
                        replica_groups=rg,
                        ins=[agin[r][hp][st : st + ln, :].opt()],
                        outs=[table[r][hp][ofs : ofs + c.NCORES * ln, :].opt()],
                    )
                    ofs += c.NCORES * ln

            def prop(r, hop, after2=None):
                """hop=1: P1T = L~ P0T. hop=2: fused output accumulation.

                after2() is emitted after group 2's calls so a following
                collective trigger doesn't block the gather queue head
                while its input is still being written."""
                tab = table[r][hop - 1]
                PTin = P0T[r] if hop == 1 else P1T[r]

                def issue_call(g, h):
                    b16 = call_base[(g, h)]
                    gt = GTS[(g * 2 + h) % NSLOT]
                    src_ap = tab[:, 0:H] if h == 0 else tab[:, H : 2 * H]
                    n_i = GP * 2 * NS * 128
                    _dma_gather_narrow(
                        nc.gpsimd, gt[:, 0:SUBS, :], src_ap,
                        idxsb[r][:, b16 : b16 + n_i // 16],
                        n_i, n_i, H, 2 * H,
                        queue_num=(g * 2 + h) % NQ)
                    return gt

                def do_pair(q, gts):
                    bs = slice(q * 128, (q + 1) * 128)
                    oh = ohp.tile([128, 2 * NS, 128], BF16, name="oh")
                    for h in range(2):
                        for ab in range(2):
                            col = ((q * 2 + h) * 2 + ab) * NS
                            nc.vector.tensor_tensor(
                                out=oh[:, h * NS : (h + 1) * NS,
                                       ab * 64 : ab * 64 + 64],
                                in0=dstoff[r][:, col : col + NS][:, :, None]
                                    .to_broadcast((128, NS, 64)),
                                in1=iota[:][:, None, ab * 64 : ab * 64 + 64]
                                    .to_broadcast((128, NS, 64)),
                                op=mybir.AluOpType.is_equal,
                            )
                    if debug and r == 0 and hop == 1 and q == DBGQ:
                        nc.sync.dma_start(out=dbg["dbg_gt0"].ap()[:],
                                          in_=gts[0][:, :, :])
                        nc.sync.dma_start(out=dbg["dbg_gt1"].ap()[:],
                                          in_=gts[1][:, :, :])
                        nc.sync.dma_start(out=dbg["dbg_oh"].ap()[:],
                                          in_=oh[:, :, :])
                    ql = q % GP
                    agg = psagg.tile([128, 128], F32, name="agg")
                    k = 0
                    for h in range(2):
                        base = ql * 2 * NS
                        for s in range(NS):
                            nc.tensor.matmul(
                                agg[:],
                                gts[h][:, base + 2 * s : base + 2 * s + 2, :],
                                oh[:, h * NS + s, :],
                                start=(k == 0),
                                stop=(k == 2 * NS - 1),
                            )
                            k += 1
                    # tmp = dinv * agg (combine the two useful quadrants)
                    tmp = scp.tile([H, 128], BF16, name="tmp")
                    dv = dinv[r][:, bs]
                    nc.vector.tensor_tensor(out=tmp[:, 0:64], in0=agg[0:64, 0:64],
                                            in1=dv[:, 0:64],
                                            op=mybir.AluOpType.mult)
                    nc.vector.tensor_tensor(out=tmp[:, 64:128],
                                            in0=agg[64:128, 64:128],
                                            in1=dv[:, 64:128],
                                            op=mybir.AluOpType.mult)
                    if hop == 1:
                        nc.vector.tensor_tensor(out=P1T[r][:, bs],
                                                in0=PTin[:, bs], in1=tmp[:],
                                                op=mybir.AluOpType.subtract)
                    else:
                        p2 = scp.tile([H, 128], BF16, name="p2")
                        nc.vector.tensor_tensor(out=p2[:], in0=PTin[:, bs],
                                                in1=tmp[:],
                                                op=mybir.AluOpType.subtract)
                        op_ps = psmisc.tile([H, 128], F32, name="opps", tag="misc")
                        nc.tensor.matmul(op_ps[:], Vk[:, 0:H], P0T[r][:, bs],
                                         start=True, stop=False)
                        nc.tensor.matmul(op_ps[:], Vk[:, H : 2 * H],
                                         P1T[r][:, bs], start=False, stop=False)
                        nc.tensor.matmul(op_ps[:], Vk[:, 2 * H : 3 * H], p2[:],
                                         start=False, stop=True)
                        if r == 0:
                            nc.vector.tensor_copy(out=outaccT[:, bs],
                                                  in_=op_ps[:])
                        else:
                            nc.vector.tensor_add(out=outaccT[:, bs],
                                                 in0=outaccT[:, bs],
                                                 in1=op_ps[:])
                            final_pair(q)

                LOOK = 4
                NG = c.NPAIR // GP
                gts_by_g = {}
                for g in range(min(LOOK + 1, NG)):
                    gts_by_g[g] = [issue_call(g, 0), issue_call(g, 1)]
                for g in range(NG):
                    g2 = g + LOOK + 1
                    if g2 < NG:
                        gts_by_g[g2] = [issue_call(g2, 0), issue_call(g2, 1)]
                    if g == 2 and after2 is not None:
                        after2()
                    for q in range(g * GP, (g + 1) * GP):
                        do_pair(q, gts_by_g[g])
                    gts_by_g.pop(g)

            def final_pair(q):
                bs = slice(q * 128, (q + 1) * 128)
                tp = psmisc.tile([128, H], BF16, name="tpo", tag="misc")
                nc.tensor.transpose(tp[:], outaccT[:, bs], identb[:H, :H])
                t = scp.tile([128, H], F32, name="tf")
                nc.vector.tensor_tensor(out=t[:], in0=tp[:], in1=b3bc[:],
                                        op=mybir.AluOpType.add)
                so = stgp.tile([128, H], F32, name="so")
                nc.vector.scalar_tensor_tensor(
                    out=so[:], in0=t[:], scalar=0.01, in1=t[:],
                    op0=mybir.AluOpType.mult, op1=mybir.AluOpType.max)
                nc.sync.dma_start(out=out_t.ap()[bs, :], in_=so[:])

            mlp(0)
            write_scaled(P0T[0], 0, agin[0][0])
            allgather(0, 0)
            deferred_loads()
            if debug:
                nc.sync.dma_start(out=dbg["dbg_P0T"].ap()[:], in_=P0T[0][:])
                nc.sync.dma_start(out=dbg["dbg_tab"].ap()[:],
                                  in_=table[0][0][:])
            mlp(1)
            write_scaled(P0T[1], 1, agin[1][0])
            prop(0, 1, after2=lambda: allgather(1, 0))
            if debug:
                nc.sync.dma_start(out=dbg["dbg_P1T"].ap()[:], in_=P1T[0][:])
            write_scaled(P1T[0], 0, agin[0][1])
            prop(1, 1, after2=lambda: allgather(0, 1))
            write_scaled(P1T[1], 1, agin[1][1])
            prop(0, 2, after2=lambda: allgather(1, 1))
            prop(1, 2)

    nc.compile()
    return nc


def _prepare(inputs, cfg):
    c = cfg
    H = c.H
    W3 = inputs["W3"]
    V = np.zeros((H, 3 * H), np.float32)
    for k in range(3):
        acc = np.zeros((H, H), np.float64)
        for i in range(3):
            acc += THETAS[i][k] * W3[i * H : (i + 1) * H].astype(np.float64)
        V[:, k * H : (k + 1) * H] = acc.astype(np.float32)

    srcs = [np.asarray(inputs["src_r1"]).astype(np.int64),
            np.asarray(inputs["src_r2"]).astype(np.int64)]
    dsts = [np.asarray(inputs["dst_r1"]).astype(np.int64),
            np.asarray(inputs["dst_r2"]).astype(np.int64)]

    # Parity-constrained balance: node n only takes a position p with
    # p%2 == n%2, so a source's table column-half (p%2) is known up front.
    POS = np.zeros(c.N, np.int64)
    H1 = np.zeros(c.N, np.int64)
    for it in range(2):
        hsrc = [(srcs[r] % 2) if it == 0 else H1[srcs[r]] for r in range(2)]
        for core in range(c.NCORES):
            cnt4 = np.zeros((c.NPC, 4), np.int64)
            for r in range(2):
                m = (dsts[r] // c.NPC) == core
                d_loc = dsts[r][m] - core * c.NPC
                np.add.at(cnt4, (d_loc, 2 * r + hsrc[r][m]), 1)
            gids = np.arange(core * c.NPC, (core + 1) * c.NPC)
            pos = _greedy_balance(cnt4, (gids % 2) if it == 0 else H1[gids],
                                  c, hard=(it == 1))
            POS[gids] = core * c.NPAD + pos
        H1 = POS % 2

    # node global padded position -> table row / column-half.
    # Table layout (chunked AllGather): [8 x rows 0:1536 | 8 x rows 1536:TPC]
    loc = POS % c.NPAD
    core_of = POS // c.NPAD
    lr = (loc // 128) * c.BLK + (loc % 128) // 2
    SROW = np.where(lr < 1536, core_of * 1536 + lr,
                    c.NCORES * 1536 + core_of * (c.TPC - 1536) + (lr - 1536))
    SH = loc % 2

    rels = []
    LH = 128
    for r in range(2):
        slots_pc, dinv, lh = _prep_relation(srcs[r], dsts[r], c, POS, SROW, SH)
        rels.append((slots_pc, dinv))
        LH = max(LH, lh)
    NS = -(-LH // 128)

    # relation 0's prop runs first: its first 2*NSLOT(=24) calls must
    # fully initialize the GTS slots (see _finalize_relation docstring)
    fin = [_finalize_relation(rels[r][0], c, NS, skip_calls=24 if r == 0 else 0)
           for r in range(2)]

    iota = np.broadcast_to(np.arange(128, dtype=np.float32), (128, 128))
    ident = np.eye(128, dtype=np.float32)

    in_maps = []
    for core in range(c.NCORES):
        m = {}
        lpos_all = POS[core * c.NPC : (core + 1) * c.NPC] - core * c.NPAD
        dvT = np.zeros((2, H, c.NPAD), np.float32)
        for r in range(2):
            idx_packed, do = fin[r][core]
            m[f"idx{r}"] = idx_packed
            m[f"dstoff{r}"] = do
            feat = np.asarray(inputs["feat_r1" if r == 0 else "feat_r2"],
                              np.float32)
            ft = np.zeros((c.F, c.NPAD), np.float32)
            ft[:, lpos_all] = feat[core * c.NPC : (core + 1) * c.NPC].T
            m[f"featT{r}"] = ft.astype(ml_dtypes.bfloat16)
            dvT[r][:, lpos_all] = np.broadcast_to(
                rels[r][1][core * c.NPC : (core + 1) * c.NPC], (H, c.NPC))
            suf = "_r1" if r == 0 else "_r2"
            m[f"W1_{r}"] = np.asarray(inputs[f"W1{suf}"],
                                      np.float32).astype(ml_dtypes.bfloat16)
            m[f"W2_{r}"] = np.asarray(inputs[f"W2{suf}"],
                                      np.float32).astype(ml_dtypes.bfloat16)
            m[f"b1_{r}"] = np.asarray(inputs[f"b1{suf}"], np.float32).reshape(H, 1)
            m[f"b2_{r}"] = np.asarray(inputs[f"b2{suf}"], np.float32).reshape(H, 1)
        m["dinv0"] = dvT[0].astype(ml_dtypes.bfloat16)
        m["dinv1"] = dvT[1].astype(ml_dtypes.bfloat16)
        m["dinvC0"] = np.ascontiguousarray(dvT[0][0].reshape(c.NPAIR, 128).T)
        m["dinvC1"] = np.ascontiguousarray(dvT[1][0].reshape(c.NPAIR, 128).T)
        m["Vk"] = V.astype(ml_dtypes.bfloat16)
        m["b3bc"] = np.broadcast_to(
            2.0 * np.asarray(inputs["b3"], np.float32), (128, H)).copy()
        m["identb"] = ident.astype(ml_dtypes.bfloat16)
        m["iota"] = iota.astype(ml_dtypes.bfloat16).copy()
        in_maps.append(m)
    return in_maps, NS, POS


def _dma_gather_narrow(gp, out_ap, in_ap, idxs_ap, num_idxs, num_idxs_reg,
                       elem_size, elem_step, queue_num=0):
    """bass.BassGpSimd.dma_gather clone allowing elem_size_bytes % 256 != 0.

    The Q7 kernel (dma_gather.cpp gen_descs, non-transpose HBM path) supports
    any payload length; only the row STRIDE must encode as stride_bytes_256.
    Used to gather 128B bf16 rows from a 256B-strided table.
    """
    import concourse.ap_utils as ap_utils
    assert idxs_ap.dtype == I16
    assert in_ap.dtype == out_ap.dtype
    assert in_ap.space == bass.MemorySpace.DRAM
    assert idxs_ap.space == bass.MemorySpace.SBUF
    assert out_ap.space == bass.MemorySpace.SBUF
    assert ap_utils.ap_is_contiguous(out_ap.ap[1:])
    assert ap_utils.ap_is_contiguous(idxs_ap.ap[1:])
    assert in_ap.ap[0][0] == elem_step
    assert in_ap.ap[-1][1] == elem_size
    assert out_ap.ap[-1][1] == elem_size
    assert out_ap.ap[0][1] * out_ap.ap[1][1] * 1 >= num_idxs
    stride_bytes = elem_step * mybir.dt.size(in_ap.dtype)
    assert stride_bytes % 256 == 0 and stride_bytes // 256 < 256
    _in_ap = gp.lower_ap_dma(in_ap, for_custom_bir_dma=True)
    _idxs_ap = gp.lower_ap(idxs_ap)
    _out_ap = gp.lower_ap(out_ap)
    return gp.add_instruction(
        mybir.InstDMAGatherAnt(
            name=gp.bass.get_next_instruction_name(),
            ins=[*_in_ap, _idxs_ap, gp.lower_val_access(gp.to_reg(num_idxs_reg))],
            outs=[_out_ap],
            transpose=False,
            num_idxs=num_idxs,
            elem_size=elem_size,
            stride_bytes_256=stride_bytes // 256,
            gen_mode=0,
            single_packet=True,
            queue_num=queue_num,
            sbuf_tokens_per_rank=0,
            sbuf_free_dim_per_rank=0,
            sbuf_free_dim_pad_per_rank=0,
            sbuf_byte_offset=0,
        )
    )


_CACHE = {}


def _install_profile_shim():
    """Provide antenv.axon_hooks (missing in this image) so trace=True works."""
    try:
        from antenv.axon_hooks import get_axon_ntff_profile_hook  # noqa: F401
        return
    except ImportError:
        pass
    import types

    import antenv
    try:
        from trn_agent_boot.trn_boot import _ntff_profile_via_ctypes
        hook = _ntff_profile_via_ctypes("/opt/axon/libaxon_pjrt.so")
    except Exception:
        hook = None
    mod = types.ModuleType("antenv.axon_hooks")
    mod._hook = hook
    mod.get_axon_ntff_profile_hook = lambda: mod._hook

    def _set(h):
        mod._hook = h

    mod.set_axon_ntff_profile_hook = _set
    sys.modules["antenv.axon_hooks"] = mod
    antenv.axon_hooks = mod


def _run(inputs, trace=False, **kw):
    if trace:
        _install_profile_shim()
    cfg = Cfg(N=int(np.asarray(inputs["feat_r1"]).shape[0]))
    in_maps, NS, POS = _prepare(inputs, cfg)
    key = (cfg.N, NS, bool(kw.pop("debug", False)))
    if key not in _CACHE:
        _CACHE[key] = _build(cfg, NS, debug=key[2])
    nc = _CACHE[key]
    res = run_bass_kernel_spmd(nc, in_maps, core_ids=list(range(cfg.NCORES)),
                               trace=trace, **kw)
    outs = []
    for core in range(cfg.NCORES):
        lpos = POS[core * cfg.NPC : (core + 1) * cfg.NPC] - core * cfg.NPAD
        outs.append(np.asarray(res.results[core]["out"])[lpos])
    full = np.concatenate(outs, axis=0)
    return full, res


def kernel(**inputs):
    full, _ = _run(inputs, trace=False)
    return full

